# revision 37
# baseline (speedup 1.0000x reference)
"""DeepseekV2 MoE layer on 8 Trainium2 NeuronCores.

Strategy (expert-parallel, per the sharding hint):
  - Router gate + grouped top-k computed on host (0.03% of module FLOPs);
    it determines the dispatch, which IS the input sharding.
  - 16 routed experts on 8 cores via three SPMD slots per core: the 8
    largest experts in slot A (capacity alpha), the 8 smallest in slot B
    (beta), and each expert's overflow beyond its slot capacity in a small
    slot C (gamma) on some core.  Capacities are chosen by search to
    minimize alpha+beta+gamma, the padded column count every core pays;
    slot C rides interleaved inside slot A's phases so its weight stream
    amortizes over the long window.
  - Shared-expert MLP is data-parallel over tokens: each core runs
    T/8 = 512 tokens through the full shared MLP.
  - Matmuls run as fp8(e4m3) DoubleRow passes over hi/lo splits of both
    operands.  Per (slot, 256-token chunk, matmul group) a LEVEL is chosen:
      0 = 3-term exact   (1.5 passes/k-slice): hi.hi + both cross terms
      1 = 2-term         (1.0): drops W_hi.X_lo  (weights-exact)
      2 = pure hi.hi     (0.5): also drops W_lo.X_hi
    Each expert's token columns are sorted by routing weight ASCENDING with
    padding first, so cheap levels land where the output barely depends on
    them; _plan_schemes greedily buys the highest cycles-per-variance steps
    until the predicted full-output l2 reaches TARGET=1.96e-2 (gate 2e-2;
    unit variances per group were measured against the f32 reference, and
    the numpy error model reproduces hardware l2 to ~1e-6).
  - f32 PSUM accumulation; bf16 outputs (combined in f32 on host).
"""

import sys

sys.path.insert(0, "/opt/trn_rl_repo")

import copy

import ml_dtypes
import numpy as np

import concourse.bass as bass
import concourse.mybir as mybir
import concourse.tile as tile
from concourse.bass_utils import run_bass_kernel_spmd

DT = mybir.dt
F8 = ml_dtypes.float8_e4m3
BF16 = ml_dtypes.bfloat16
DR = mybir.MatmulPerfMode.DoubleRow

T, D, E, I = 4096, 2048, 16, 1024
TOP_K, N_GROUP, TOPK_GROUP = 4, 4, 2
ROUTED_SCALE = 2.5
SHARED_I = 2048
N_CORES = 8
P = 128
NCH = 256  # token chunk (DoubleRow moving free = 2*NCH = 512 max)

SX = 16.0  # x scale into e4m3
SW = 512.0  # weight scale into e4m3
SH = 8.0  # h scale into e4m3
CU = SH / (SX * SW * SX * SW)  # ps_u -> u*SH/(SX*SW)
CY = 1.0 / (SH * SW)  # down psum descale


# ---------------------------------------------------------------- wait split
def _split_excess_waits(nc, limit=1):
    """This walrus build rejects >1 sync-wait command per instruction.
    Move excess waits onto fresh same-engine NOPs inserted just before."""
    template = bass.Bass(target_bir_lowering=False).sync.nop(nofuse=True).ins
    ctr = 0
    for bb in nc.main_func.blocks:
        out = []
        changed = False
        for ins in bb.instructions:
            si = ins.sync_info
            if si is not None and si.on_wait and len(si.on_wait) > limit:
                waits = list(si.on_wait)
                for w in waits[:-limit]:
                    ctr += 1
                    nop = copy.deepcopy(template)
                    nop.name = f"I-wsplit-{ctr}"
                    nop.engine = ins.engine
                    nop.bass_nofuse = True
                    nop.sync_info = mybir.SyncInfo(on_wait=[w], on_update=[])
                    nc.register_instruction(nop, overwrite=True)
                    out.append(nop)
                ins.sync_info = mybir.SyncInfo(
                    on_wait=waits[-limit:], on_update=list(si.on_update)
                )
                changed = True
            out.append(ins)
        if changed:
            bb.instructions = out
    return ctr


# ---------------------------------------------------------------- routing
def _gate_logits(x, gate_w):
    # Match the reference's jax-f32 CPU matmul as closely as possible.
    try:
        import jax
        import jax.numpy as jnp

        cpu = jax.devices("cpu")[0]
        with jax.default_device(cpu):
            return np.asarray(jnp.matmul(jnp.asarray(x), jnp.asarray(gate_w)))
    except Exception:
        return (x @ gate_w).astype(np.float32)


def _route(x, gate_w, e_bias):
    logits = _gate_logits(x, gate_w)  # [T, E] f32
    scores = (1.0 / (1.0 + np.exp(-logits))).astype(np.float32)
    sfc = scores + e_bias[None, :]
    grp = sfc.reshape(T, N_GROUP, E // N_GROUP)
    group_scores = np.sort(grp, axis=-1)[:, :, -2:].sum(-1)  # [T, G]
    group_idx = np.argsort(-group_scores, axis=-1, kind="stable")[:, :TOPK_GROUP]
    group_mask = np.zeros((T, N_GROUP), bool)
    group_mask[np.arange(T)[:, None], group_idx] = True
    expert_mask = np.repeat(group_mask, E // N_GROUP, axis=1)
    masked = np.where(expert_mask, sfc, -np.inf)
    topk_idx = np.argsort(-masked, axis=-1, kind="stable")[:, :TOP_K]  # [T, 4]
    topk_w = np.take_along_axis(scores, topk_idx, axis=1)
    topk_w = topk_w / topk_w.sum(axis=1, keepdims=True)
    return topk_idx.astype(np.int64), topk_w.astype(np.float32)


# ---------------------------------------------------------------- program
_PROGRAM_CACHE = {}


def _mm3(nc, ps, wt, xt, nk, tok, sz, first, last, level=0, pm=False):
    """fp8 DoubleRow contraction over nk k-slices of 128.

    wt: stationary tile [P, nk, 2, P] with slot0=hi, slot1=lo.
    xt: moving tile [P, nk, 2, C] with slot0=lo, slot1=hi — or, with
    pm=True (plane-major), [P, 2, nk, C] with plane0=lo, plane1=hi so the
    hi plane can be DMA'd compactly ahead of the lo plane.
    ps: psum [P, NCH] (use [:, :sz]); tok = token offset into xt.

    level 0: 3-term exact (hi.hi pairs + per-slice full cross terms);
    level 1: weights-exact 2-term (hi.hi + lo.hi pairs), drops w_hi.x_lo;
    level 2: pure hi.hi, additionally drops w_lo.x_hi.
    """

    def mv_hi(j2):  # (2 k-slices, sz) hi rows
        if pm:
            return xt[:, 1, 2 * j2 : 2 * j2 + 2, tok : tok + sz]
        return xt[:, 2 * j2 : 2 * j2 + 2, 1, tok : tok + sz]

    def mv_cross(k):  # ((lo,hi), sz) of one k-slice
        if pm:
            return xt[:, :, k, tok : tok + sz]
        return xt[:, k, :, tok : tok + sz]

    # hi*hi over k-slice pairs
    for j in range(nk // 2):
        nc.tensor.matmul(
            ps[:, :sz],
            wt[:, 2 * j : 2 * j + 2, 0, :],
            mv_hi(j),
            start=(first and j == 0),
            stop=(last and level == 2 and j == nk // 2 - 1),
            perf_mode=DR,
        )
    if level == 0:
        # full cross terms: (w_hi, w_lo) x (x_lo, x_hi) per k-slice
        for k in range(nk):
            nc.tensor.matmul(
                ps[:, :sz],
                wt[:, k, :, :],
                mv_cross(k),
                start=False,
                stop=(last and k == nk - 1),
                perf_mode=DR,
            )
    elif level == 1:
        # weights-exact residual: (w_lo, w_lo') x (x_hi, x_hi') slice pairs
        for j in range(nk // 2):
            nc.tensor.matmul(
                ps[:, :sz],
                wt[:, 2 * j : 2 * j + 2, 1, :],
                mv_hi(j),
                start=False,
                stop=(last and j == nk // 2 - 1),
                perf_mode=DR,
            )


def _load_xt(nc, pools, sp, first=False):
    """Emit a spec's x (and wr) loads; idempotent via sp['xt_t']."""
    for th in _xt_load_thunks(nc, pools, sp, first):
        th()


def _xt_load_thunks(nc, pools, sp, first=False):
    """Create the spec's x/wr tiles and return one thunk per DMA, so a
    caller can dribble the emissions between other queue traffic."""
    (xt_pool, w1_pool, w2_pool, g_pool, h_pool, y_pool, wr_pool, sg_pool,
     tmp_pool, ps_gu, ps_dn) = pools
    if "xt_t" in sp:
        return []
    n_d = D // P
    xt_q = nc.gpsimd if sp["bulk_q"] else nc.sync
    C = sp["C"]
    thunks = []
    if first:
        # chunk- and plane-major tile [P, n_chunk, 2, n_d, NCH]: per chunk
        # the hi plane (512KB) lands first so hi.hi passes start early; the
        # lo plane (cross terms) streams behind it
        n_ch = C // NCH
        sp["xt_t"] = xt_pool.tile([P, n_ch, 2, n_d, NCH], DT.float8e4,
                                  name=sp["xt_name"])
        sp["chunk_major"] = True
        for ci in range(n_ch):
            plans = [(1, 0, 4), (1, 4, 16), (0, 0, 8), (0, 8, 16)] if ci == 0 \
                else [(1, 0, 16), (0, 0, 16)]
            for pl, a, b in plans:
                thunks.append(lambda ci=ci, pl=pl, a=a, b=b: xt_q.dma_start(
                    sp["xt_t"][:, ci, pl, a:b, :], sp["xt_h"][ci][:, pl, a:b]))
        return thunks
    sp["xt_t"] = xt_pool.tile([P, n_d, 2, C], DT.float8e4, name=sp["xt_name"])
    if C <= NCH:
        thunks.append(lambda: xt_q.dma_start(sp["xt_t"][:], sp["xt_h"][:, :]))
    else:
        for a, b in zip(range(n_d), range(1, n_d + 1)):
            thunks.append(lambda a=a, b=b: xt_q.dma_start(
                sp["xt_t"][:, a:b, :, :], sp["xt_h"][:, a:b]))
    if sp["apply_wr"]:
        sp["wr_t"] = wr_pool.tile([P, C], DT.float32, name="wr")
        thunks.append(lambda: xt_q.dma_start(sp["wr_t"][:], sp["wr_h"][:, :]))
    return thunks


def _w1_order(n_specs, n_h):
    # merged w1-slice order: per pair ih, each spec's (gate ih, up ih+n_h)
    order = []
    for ih in range(n_h):
        for si in range(n_specs):
            order += [(si, ih, 0), (si, ih + n_h, 1)]
    return order


def _emit_experts(nc, tc, pools, specs, twoI, schemes, first=False,
                  prefetch=(), last=False):
    """Emit 1-2 experts processed interleaved (pair-by-pair, then d2-by-d2).

    Each spec: dict(xt_h, w1_h, w2_h, wr_h, y_h, C, apply_wr, bulk_q).
    A small companion expert rides inside the big one's phases so its
    weight stream amortizes over the long window instead of starving a
    short trailing phase.
    """
    n_d = D // P  # 16 contraction slices over D
    n_i = twoI // P  # gate_up output tiles
    n_h = n_i // 2  # h tiles (= I_/128)

    (xt_pool, w1_pool, w2_pool, g_pool, h_pool, y_pool, wr_pool, sg_pool,
     tmp_pool, ps_gu, ps_dn) = pools

    for sp in specs:
        sp["chunks"] = [(o, min(NCH, sp["C"] - o)) for o in range(0, sp["C"], NCH)]

    order = _w1_order(len(specs), n_h)

    # All w1 loads go on the Pool queue.  Two effects: they never queue
    # behind the previous expert's w2 stream on SP, and — because the queue
    # is in-order and the w1 buffer rotation WAR-throttles it to compute
    # pace — the x bulk loads emitted after them are naturally delayed into
    # the mid-gate_up window, away from the congested phase boundaries.
    def load_w1(si, i):
        t = w1_pool.tile([P, n_d, 2, P], DT.float8e4, name="w1s")
        nc.gpsimd.dma_start(t[:], specs[si]["w1_h"][i])
        return t

    n_pre = 6 if first else 3
    w1_tiles = {j: load_w1(order[j][0], order[j][1]) for j in range(n_pre)}

    # whole-expert X tile [P, k-slice, (lo,hi), tok].  First expert: chunky
    # loads (SP-issue rate is the cold-start limiter).  Later experts: per-d
    # slices on the Pool queue, so each transfer is short and never
    # head-of-line-blocks the latency-critical weight-slice stream on the
    # shared DMA engines.
    for sp in specs:
        _load_xt(nc, pools, sp, first)
        sp["h_t"] = h_pool.tile([P, n_h, 2, sp["C"]], DT.float8e4, name="hil")
        sp["gt"] = {}

    # next experts' x bulk loads dribble into the queue mid-gate_up, two
    # DMAs per pair-step, so they never monopolize the DMA engines against
    # this phase's own weight stream
    pf_thunks = []

    # gate_up: (gate i, up i+n_h) pairs so gate tiles die quickly
    for j, (si, i, half) in enumerate(order):
        if j == n_pre:
            for psp in prefetch:
                pf_thunks += _xt_load_thunks(nc, pools, psp)
        if j >= n_pre:
            for _ in range(2):
                if pf_thunks:
                    pf_thunks.pop(0)()
        sp = specs[si]
        ih = i if half == 0 else i - n_h
        w1s = w1_tiles.pop(j)
        if j + n_pre < len(order):
            nj = j + n_pre
            w1_tiles[nj] = load_w1(order[nj][0], order[nj][1])
        xt_t, h_t = sp["xt_t"], sp["h_t"]
        grp = "g" if half == 0 else "u"
        for ci, (off, sz) in enumerate(sp["chunks"]):
            lvl = schemes.get((sp["slot"], ci, grp), 0)
            ps = ps_gu.tile([P, NCH], DT.float32, name="psg")
            if sp.get("chunk_major"):
                _mm3(nc, ps, w1s, xt_t[:, ci], n_d, 0, sz, True, True, lvl,
                     pm=True)
            else:
                _mm3(nc, ps, w1s, xt_t, n_d, off, sz, True, True, lvl)
            if half == 0:
                sg = sg_pool.tile([P, NCH], DT.float32, name="sg")
                nc.scalar.activation(
                    sg[:, :sz], ps[:, :sz],
                    mybir.ActivationFunctionType.Sigmoid,
                    scale=1.0 / (SX * SW),
                )
                gt = g_pool.tile([P, NCH], DT.float32, name="gt")
                nc.vector.tensor_mul(gt[:, :sz], ps[:, :sz], sg[:, :sz])
                sp["gt"][ci] = gt
            else:
                us = tmp_pool.tile([P, NCH], DT.float32, name="us")
                nc.vector.tensor_scalar_mul(us[:, :sz], ps[:, :sz], CU)
                th = tmp_pool.tile([P, NCH], DT.float32, name="th")
                nc.vector.tensor_mul(th[:, :sz], sp["gt"][ci][:, :sz], us[:, :sz])
                nc.scalar.copy(h_t[:, ih, 1, off : off + sz], th[:, :sz])
                if schemes.get((sp["slot"], ci, "d"), 0) == 0:
                    # h_lo only needed where the down-proj runs 3-term
                    df = tmp_pool.tile([P, NCH], DT.float32, name="df")
                    nc.vector.tensor_sub(
                        df[:, :sz], th[:, :sz], h_t[:, ih, 1, off : off + sz]
                    )
                    nc.scalar.copy(h_t[:, ih, 0, off : off + sz], df[:, :sz])

    for th in pf_thunks:
        th()

    # down projection; w2 slices prefetched 3 merged-steps ahead on SP
    dorder = [(si, d2) for d2 in range(D // P) for si in range(len(specs))]

    def load_w2(j, si, d2):
        t = w2_pool.tile([P, n_h, 2, P], DT.float8e4, name="w2s")
        nc.sync.dma_start(t[:], specs[si]["w2_h"][d2])
        return t

    w2_tiles = {j: load_w2(j, *dorder[j]) for j in range(3)}
    for j, (si, d2) in enumerate(dorder):
        sp = specs[si]
        w2s = w2_tiles.pop(j)
        if j + 3 < len(dorder):
            w2_tiles[j + 3] = load_w2(j + 3, *dorder[j + 3])
        chunks, h_t, C = sp["chunks"], sp["h_t"], sp["C"]
        ys = y_pool.tile([P, C], DT.bfloat16, name="ys")
        # one output DMA per row, spread over three queues: HWDGE descriptor
        # generation (one shared unit, ~630ns per DMA) is the down-phase
        # bottleneck, so small rows go to the Pool SWDGE instead
        if C <= NCH:
            y_q = nc.gpsimd
        else:
            y_q = nc.scalar if d2 % 2 else nc.sync
        for ci, (off, sz) in enumerate(chunks):
            lvl = schemes.get((sp["slot"], ci, "d"), 0)
            ps = ps_dn.tile([P, NCH], DT.float32, name="psd")
            _mm3(nc, ps, w2s, h_t, n_h, off, sz, True, True, lvl)
            if sp["apply_wr"]:
                nc.vector.tensor_mul(ys[:, off : off + sz], ps[:, :sz],
                                     sp["wr_t"][:, off : off + sz])
            else:
                nc.scalar.mul(ys[:, off : off + sz], ps[:, :sz], CY)
            if last and j == len(dorder) - 1:
                # final row of the program: drain per-chunk; spread over the
                # two HWDGE queues AND the Pool SWDGE so descriptor
                # generation (625ns apiece, serialized per unit) overlaps
                q = (nc.sync, nc.gpsimd, nc.scalar, nc.gpsimd)[ci % 4]
                q.dma_start(sp["y_h"][d2 * P : (d2 + 1) * P, off : off + sz],
                            ys[:, off : off + sz])
        if not (last and j == len(dorder) - 1):
            y_q.dma_start(sp["y_h"][d2 * P : (d2 + 1) * P, :], ys[:])


def _build_program(C1, C2, C3, skey=()):
    key = (C1, C2, C3, skey)
    if key in _PROGRAM_CACHE:
        return _PROGRAM_CACHE[key]
    schemes = dict(skey)

    nc = bass.Bass(target_bir_lowering=False)
    TS = T // N_CORES  # shared tokens per core
    n_d = D // P

    xt1 = nc.dram_tensor("xt1", [P, n_d, 2, C1], DT.float8e4, kind="ExternalInput")
    xt2 = nc.dram_tensor("xt2", [P, n_d, 2, C2], DT.float8e4, kind="ExternalInput")
    # shared x is chunk-major so the cold start only waits on chunk 0's
    # columns (1MB) instead of the whole tile before the first matmul
    xts = nc.dram_tensor("xts", [TS // NCH, P, 2, n_d, NCH], DT.float8e4,
                         kind="ExternalInput")
    w1a = nc.dram_tensor("w1a", [2 * I // P, P, n_d, 2, P], DT.float8e4, kind="ExternalInput")
    w2a = nc.dram_tensor("w2a", [D // P, P, I // P, 2, P], DT.float8e4, kind="ExternalInput")
    w1b = nc.dram_tensor("w1b", [2 * I // P, P, n_d, 2, P], DT.float8e4, kind="ExternalInput")
    w2b = nc.dram_tensor("w2b", [D // P, P, I // P, 2, P], DT.float8e4, kind="ExternalInput")
    ws1 = nc.dram_tensor("ws1", [2 * SHARED_I // P, P, n_d, 2, P], DT.float8e4, kind="ExternalInput")
    ws2 = nc.dram_tensor("ws2", [D // P, P, SHARED_I // P, 2, P], DT.float8e4, kind="ExternalInput")
    wr1 = nc.dram_tensor("wr1", [P, C1], DT.float32, kind="ExternalInput")
    wr2 = nc.dram_tensor("wr2", [P, C2], DT.float32, kind="ExternalInput")
    y1 = nc.dram_tensor("y1", [D, C1], DT.bfloat16, kind="ExternalOutput")
    y2 = nc.dram_tensor("y2", [D, C2], DT.bfloat16, kind="ExternalOutput")
    ys = nc.dram_tensor("ys", [D, TS], DT.bfloat16, kind="ExternalOutput")
    if C3:
        xt3 = nc.dram_tensor("xt3", [P, n_d, 2, C3], DT.float8e4, kind="ExternalInput")
        w1c = nc.dram_tensor("w1c", [2 * I // P, P, n_d, 2, P], DT.float8e4, kind="ExternalInput")
        w2c = nc.dram_tensor("w2c", [D // P, P, I // P, 2, P], DT.float8e4, kind="ExternalInput")
        wr3 = nc.dram_tensor("wr3", [P, C3], DT.float32, kind="ExternalInput")
        y3 = nc.dram_tensor("y3", [D, C3], DT.bfloat16, kind="ExternalOutput")

    # gate tiles of a pair stay live across all of that pair's chunks: the
    # pool must hold one buffer per chunk or the rotation WAR-deadlocks
    max_chunks = max(-(-c // NCH) for c in (C1, C2, T // N_CORES))
    with tile.TileContext(nc) as tc:
        with (
            tc.tile_pool(name="xt", bufs=1) as xt_pool,
            tc.tile_pool(name="w1p", bufs=6) as w1_pool,
            tc.tile_pool(name="w2p", bufs=4) as w2_pool,
            tc.tile_pool(name="gp", bufs=max(10, max_chunks + 3)) as g_pool,
            tc.tile_pool(name="hp", bufs=2) as h_pool,
            tc.tile_pool(name="yp", bufs=3) as y_pool,
            tc.tile_pool(name="wrp", bufs=2) as wr_pool,
            tc.tile_pool(name="sgp", bufs=4) as sg_pool,
            tc.tile_pool(name="tmp", bufs=4) as tmp_pool,
            tc.tile_pool(name="psgu", bufs=4, space="PSUM") as ps_gu,
            tc.tile_pool(name="psdn", bufs=4, space="PSUM") as ps_dn,
        ):
            pools = (xt_pool, w1_pool, w2_pool, g_pool, h_pool, y_pool, wr_pool,
                     sg_pool, tmp_pool, ps_gu, ps_dn)

            def spec(xt_h, w1_h, w2_h, wr_h, y_h, C, apply_wr, bulk_q, xt_name,
                     slot):
                return dict(xt_h=xt_h, w1_h=w1_h, w2_h=w2_h, wr_h=wr_h,
                            y_h=y_h, C=C, apply_wr=apply_wr, bulk_q=bulk_q,
                            xt_name=xt_name, slot=slot)

            # shared first: its small x-load makes the cold-start short, and
            # the routed experts' larger input streams prefetch underneath it.
            # The small C slot rides inside expert A's phases.
            s_sh = spec(xts, ws1, ws2, None, ys, TS, False, False, "xts", "S")
            sa = [spec(xt1, w1a, w2a, wr1, y1, C1, True, True, "xt1", "A")]
            if C3:
                sa.append(spec(xt3, w1c, w2c, wr3, y3, C3, True, True, "xt3", "C"))
            s_b = spec(xt2, w1b, w2b, wr2, y2, C2, True, True, "xt2", "B")
            _emit_experts(nc, tc, pools, [s_sh], 2 * SHARED_I, schemes,
                          first=True, prefetch=sa)
            _emit_experts(nc, tc, pools, sa, 2 * I, schemes, prefetch=[s_b])
            _emit_experts(nc, tc, pools, [s_b], 2 * I, schemes, last=True)

    _split_excess_waits(nc, limit=1)
    _PROGRAM_CACHE[key] = nc
    return nc


# ---------------------------------------------------------------- packing
def _hi_lo(a, scale):
    s = (a * scale).astype(np.float32)
    hi = s.astype(F8)
    lo = (s - hi.astype(np.float32)).astype(F8)
    return hi, lo


def _pack_w(w, scale):
    """w [K, F] f32 -> [F/P, P(k-in-slice), K/P, 2(hi,lo), P(feat)] e4m3."""
    K, F = w.shape
    n_k, n_f = K // P, F // P
    hi, lo = _hi_lo(w, scale)

    def arr(a):
        return a.reshape(n_k, P, n_f, P).transpose(2, 1, 0, 3)

    out = np.empty((n_f, P, n_k, 2, P), F8)
    out[:, :, :, 0, :] = arr(hi)
    out[:, :, :, 1, :] = arr(lo)
    return np.ascontiguousarray(out)


def _pack_x(xhiT, xloT, cols):
    """xhiT/xloT [D, T] e4m3 + column index -> [P, D/P, 2(lo,hi), C]."""
    n_d = D // P
    C = len(cols)
    out = np.empty((P, n_d, 2, C), F8)
    out[:, :, 0, :] = xloT[:, cols].reshape(n_d, P, C).transpose(1, 0, 2)
    out[:, :, 1, :] = xhiT[:, cols].reshape(n_d, P, C).transpose(1, 0, 2)
    return np.ascontiguousarray(out)


def _cap(n):
    # exact capacity; keep a small floor so degenerate routings stay sane
    return max(P, int(n))


def _plan_slots(counts):
    """Choose slot capacities (alpha, beta, gamma) and the token split.

    Slot A holds the 8 largest experts capped at alpha, slot B the 8
    smallest capped at beta; each expert's overflow goes to one (or more) of
    the 8 per-core C slots of capacity gamma.  Minimizing alpha+beta+gamma
    minimizes the padded per-core column count the SPMD program pays.
    """
    by = np.argsort(-counts, kind="stable")
    big, small = by[:N_CORES], by[N_CORES:]
    cb, cs = counts[big], counts[small]
    def min_gamma(a, b):
        exc = np.concatenate([np.maximum(0, cb - a), np.maximum(0, cs - b)])
        pos = exc[exc > 0]
        if len(pos) == 0:
            return 0
        lo, hi = 1, int(pos.max())
        while lo < hi:
            mid = (lo + hi) // 2
            if np.ceil(pos / mid).sum() <= N_CORES:
                hi = mid
            else:
                lo = mid + 1
        if np.ceil(pos / lo).sum() > N_CORES:
            return None
        return lo

    def scan(a_rng, b_rng):
        best = None
        for a in a_rng:
            for b in b_rng:
                g = min_gamma(a, b)
                if g is None:
                    continue
                tot = a + b + g
                if best is None or tot < best[0]:
                    best = (tot, a, b, g)
        return best

    a_hi, b_hi = int(cb.max()), int(cs.max())
    best = scan(range(max(P, a_hi - 512), a_hi + 1, 8),
                range(max(P, b_hi - 512), b_hi + 1, 8))
    _, a0, b0, _ = best
    best = scan(range(max(P, a0 - 8), min(a_hi, a0 + 8) + 1),
                range(max(P, b0 - 8), min(b_hi, b0 + 8) + 1))
    _, alpha, beta, gamma = best
    # C segments: (expert_id, offset_into_expert_token_list, length)
    segs = []
    for e, cap in [(int(e), alpha) for e in big] + [(int(e), beta) for e in small]:
        exc = int(counts[e]) - cap
        off = cap
        while exc > 0:
            take = min(exc, gamma)
            segs.append((e, off, take))
            off += take
            exc -= take
    assert len(segs) <= N_CORES
    return big, small, alpha, beta, gamma, segs


def _plan_schemes(wlists, C1, C2, C3):
    """Greedy error-budget allocator: per-(slot,chunk,group) matmul levels.

    Each step (exact->2term or 2term->pure) on a routed chunk saves
    32*sz PE cycles and adds VU[g] * (that chunk's share of sum(wr^2))
    of squared-l2 error; columns are wr-ascending with padding first, so
    early chunks are cheap.  Unit variances were measured against the
    f32 reference with single-group probes; pure = 2x 2term (verified).
    Budget keeps the predicted full-output l2 under TARGET (gate 2e-2).

    wlists: slot -> list of per-core padded wr arrays (wr=0 padding)."""
    VU = {"g": 322.9e-6, "u": 301.2e-6, "d": 313.1e-6}
    VS = {"g": 405.6e-6, "u": 378.5e-6, "d": 386.4e-6}
    FLOOR2 = 2.5683e-3 ** 2
    TARGET = 1.979e-2
    W2ALL = max(sum(float((w * w).sum()) for ws in wlists.values() for w in ws),
                1e-30)
    items = []
    for slot, C in (("A", C1), ("B", C2), ("C", C3)):
        if not C:
            continue
        for ci in range(-(-C // NCH)):
            off = ci * NCH
            sz = min(NCH, C - off)
            share = sum(float((w[off : off + sz] ** 2).sum())
                        for w in wlists[slot]) / W2ALL
            for g in ("g", "u", "d"):
                var = max(VU[g] * share, 1e-12)
                for step in (1, 2):
                    items.append((32.0 * sz / var, 32.0 * sz, var, (slot, ci, g), step))
    for g in ("g", "u", "d"):
        for step in (1, 2):
            items.append((64.0 * (T // N_CORES) / VS[g], 64.0 * (T // N_CORES),
                          VS[g], ("S", 0, g), step))
    items.sort(key=lambda t: (-t[0], t[4]))
    budget = TARGET ** 2 - FLOOR2
    taken, used = {}, 0.0
    for _ in range(2):  # second pass lets step-2 follow a same-key step-1
        for ratio, save, var, key, step in items:
            if step != taken.get(key, 0) + 1 or used + var > budget:
                continue
            taken[key] = step
            used += var
    return taken


# ---------------------------------------------------------------- kernel
def _prepare(hidden_states, gate_w, e_bias, w_gate_up, w_down, ws_gate_up, ws_down):
    x = np.asarray(hidden_states, dtype=np.float32)
    topk_idx, topk_w = _route(x, np.asarray(gate_w), np.asarray(e_bias))

    # dispatch: token lists per expert, sorted-stable by expert id
    flat_e = topk_idx.ravel()
    order = np.argsort(flat_e, kind="stable")
    pair_tok = order // TOP_K
    pair_w = (topk_w.ravel()[order] * ROUTED_SCALE).astype(np.float32)
    counts = np.bincount(flat_e, minlength=E)
    starts = np.zeros(E + 1, np.int64)
    np.cumsum(counts, out=starts[1:])

    # wr-ascending sort within each expert: low-weight tokens land in the
    # early chunks, where the allocator spends the error budget
    for e in range(E):
        sl = slice(starts[e], starts[e + 1])
        o = np.argsort(pair_w[sl], kind="stable")
        pair_tok[sl] = pair_tok[sl][o]
        pair_w[sl] = pair_w[sl][o]

    # expert -> core assignment: 8 largest in slot A, 8 smallest in slot B
    # (pairing big-with-small per core), overflow segments in slot C
    slotA, slotB_u, alpha, beta, gamma, segs = _plan_slots(counts)
    slotB = slotB_u[::-1]  # pair biggest A with smallest B
    C1 = _cap(alpha)
    C2 = _cap(beta)
    C3 = max(16, int(gamma)) if gamma else 0

    # per-core padded slot arrays, PAD-FIRST: padding columns (wr=0) sit at
    # the low-wr front so the cheap-scheme chunks absorb them for free
    TS = T // N_CORES
    core_slots = []
    wlists = {"A": [], "B": [], "C": []}
    for c in range(N_CORES):
        eA, eB = int(slotA[c]), int(slotB[c])
        segC = segs[c] if c < len(segs) else None
        slots = [("A", eA, 0, C1, C1, "xt1", "wr1"), ("B", eB, 0, C2, C2, "xt2", "wr2")]
        if C3:
            if segC is not None:
                slots.append(("C", segC[0], segC[1], segC[2], C3, "xt3", "wr3"))
            else:
                slots.append(("C", eA, 0, 0, C3, "xt3", "wr3"))
        padded = []
        for slot, e_id, off, cap, C, xt_name, wr_name in slots:
            sl = slice(starts[e_id] + off, min(starts[e_id + 1], starts[e_id] + off + cap))
            idx = pair_tok[sl]
            w = pair_w[sl]
            n_e = len(idx)
            idx_pad = np.zeros(C, np.int64)
            idx_pad[C - n_e :] = idx
            w_pad = np.zeros(C, np.float32)
            w_pad[C - n_e :] = w
            wlists[slot].append(w_pad)
            padded.append((xt_name, wr_name, idx_pad, w_pad, idx, n_e))
        core_slots.append(padded)

    schemes = _plan_schemes(wlists, C1, C2, C3)
    nc = _build_program(C1, C2, C3, tuple(sorted(schemes.items())))

    xhi, xlo = _hi_lo(x, SX)  # [T, D] e4m3
    xhiT = np.ascontiguousarray(xhi.T)  # [D, T]
    xloT = np.ascontiguousarray(xlo.T)

    ws1_p = _pack_w(np.asarray(ws_gate_up), SW)
    ws2_p = _pack_w(np.asarray(ws_down), SW)
    w_gate_up = np.asarray(w_gate_up)
    w_down = np.asarray(w_down)

    in_maps = []
    core_info = []
    zero_w1 = zero_w2 = None
    for c in range(N_CORES):
        eA, eB = int(slotA[c]), int(slotB[c])
        segC = segs[c] if c < len(segs) else None
        m = {}
        info = []
        for xt_name, wr_name, idx_pad, w_pad, idx, n_e in core_slots[c]:
            m[xt_name] = _pack_x(xhiT, xloT, idx_pad)
            m[wr_name] = np.ascontiguousarray(
                np.broadcast_to(w_pad * CY, (P, len(w_pad))))
            info.append((idx, n_e))
        m["xts"] = np.stack([
            np.ascontiguousarray(
                _pack_x(xhiT, xloT,
                        np.arange(c * TS + b * NCH, c * TS + (b + 1) * NCH)
                        ).transpose(0, 2, 1, 3))
            for b in range(TS // NCH)
        ])
        m["w1a"] = _pack_w(w_gate_up[eA], SW)
        m["w2a"] = _pack_w(w_down[eA], SW)
        m["w1b"] = _pack_w(w_gate_up[eB], SW)
        m["w2b"] = _pack_w(w_down[eB], SW)
        if C3:
            if segC is not None:
                m["w1c"] = _pack_w(w_gate_up[segC[0]], SW)
                m["w2c"] = _pack_w(w_down[segC[0]], SW)
            else:
                if zero_w1 is None:
                    zero_w1 = np.zeros((2 * I // P, P, D // P, 2, P), F8)
                    zero_w2 = np.zeros((D // P, P, I // P, 2, P), F8)
                m["w1c"] = zero_w1
                m["w2c"] = zero_w2
        m["ws1"] = ws1_p
        m["ws2"] = ws2_p
        in_maps.append(m)
        core_info.append(info)
    return nc, in_maps, core_info


def _combine(res_results, core_info):
    TS = T // N_CORES
    out = np.zeros((T, D), np.float32)
    for c in range(N_CORES):
        for (idx, n), y_name in zip(core_info[c], ("y1", "y2", "y3")):
            if n:
                # pad-first layout: real columns are the LAST n of the slot
                y = res_results[c][y_name]
                out[idx] += y[:, y.shape[1] - n :].astype(np.float32).T
        out[c * TS : (c + 1) * TS] += res_results[c]["ys"].astype(np.float32).T
    return out


def kernel(hidden_states, gate_w, e_bias, w_gate_up, w_down, ws_gate_up, ws_down):
    nc, in_maps, core_info = _prepare(
        hidden_states, gate_w, e_bias, w_gate_up, w_down, ws_gate_up, ws_down
    )
    res = run_bass_kernel_spmd(nc, in_maps, list(range(N_CORES)))
    return _combine(res.results, core_info)



# revision 39
# speedup vs baseline: 1.0011x; 1.0011x over previous
"""DeepseekV2 MoE layer on 8 Trainium2 NeuronCores.

Strategy (expert-parallel, per the sharding hint):
  - Router gate + grouped top-k computed on host (0.03% of module FLOPs);
    it determines the dispatch, which IS the input sharding.
  - 16 routed experts on 8 cores via three SPMD slots per core: the 8
    largest experts in slot A (capacity alpha), the 8 smallest in slot B
    (beta), and each expert's overflow beyond its slot capacity in a small
    slot C (gamma) on some core.  Capacities are chosen by search to
    minimize alpha+beta+gamma, the padded column count every core pays;
    slot C rides interleaved inside slot A's phases so its weight stream
    amortizes over the long window.
  - Shared-expert MLP is data-parallel over tokens: each core runs
    T/8 = 512 tokens through the full shared MLP.
  - Matmuls run as fp8(e4m3) DoubleRow passes over hi/lo splits of both
    operands.  Per (slot, 256-token chunk, matmul group) a LEVEL is chosen:
      0 = 3-term exact   (1.5 passes/k-slice): hi.hi + both cross terms
      1 = 2-term         (1.0): drops W_hi.X_lo  (weights-exact)
      2 = pure hi.hi     (0.5): also drops W_lo.X_hi
    Each expert's token columns are sorted by routing weight ASCENDING with
    padding first, so cheap levels land where the output barely depends on
    them; _plan_schemes greedily buys the highest cycles-per-variance steps
    until the predicted full-output l2 reaches TARGET=1.96e-2 (gate 2e-2;
    unit variances per group were measured against the f32 reference, and
    the numpy error model reproduces hardware l2 to ~1e-6).
  - f32 PSUM accumulation; bf16 outputs (combined in f32 on host).
"""

import sys

sys.path.insert(0, "/opt/trn_rl_repo")

import copy

import ml_dtypes
import numpy as np

import concourse.bass as bass
import concourse.mybir as mybir
import concourse.tile as tile
from concourse.bass_utils import run_bass_kernel_spmd

DT = mybir.dt
F8 = ml_dtypes.float8_e4m3
BF16 = ml_dtypes.bfloat16
DR = mybir.MatmulPerfMode.DoubleRow

T, D, E, I = 4096, 2048, 16, 1024
TOP_K, N_GROUP, TOPK_GROUP = 4, 4, 2
ROUTED_SCALE = 2.5
SHARED_I = 2048
N_CORES = 8
P = 128
NCH = 256  # token chunk (DoubleRow moving free = 2*NCH = 512 max)

SX = 16.0  # x scale into e4m3
SW = 512.0  # weight scale into e4m3
SH = 8.0  # h scale into e4m3
CU = SH / (SX * SW * SX * SW)  # ps_u -> u*SH/(SX*SW)
CY = 1.0 / (SH * SW)  # down psum descale


# ---------------------------------------------------------------- wait split
def _split_excess_waits(nc, limit=1):
    """This walrus build rejects >1 sync-wait command per instruction.
    Move excess waits onto fresh same-engine NOPs inserted just before."""
    template = bass.Bass(target_bir_lowering=False).sync.nop(nofuse=True).ins
    ctr = 0
    for bb in nc.main_func.blocks:
        out = []
        changed = False
        for ins in bb.instructions:
            si = ins.sync_info
            if si is not None and si.on_wait and len(si.on_wait) > limit:
                waits = list(si.on_wait)
                for w in waits[:-limit]:
                    ctr += 1
                    nop = copy.deepcopy(template)
                    nop.name = f"I-wsplit-{ctr}"
                    nop.engine = ins.engine
                    nop.bass_nofuse = True
                    nop.sync_info = mybir.SyncInfo(on_wait=[w], on_update=[])
                    nc.register_instruction(nop, overwrite=True)
                    out.append(nop)
                ins.sync_info = mybir.SyncInfo(
                    on_wait=waits[-limit:], on_update=list(si.on_update)
                )
                changed = True
            out.append(ins)
        if changed:
            bb.instructions = out
    return ctr


# ---------------------------------------------------------------- routing
def _gate_logits(x, gate_w):
    # Match the reference's jax-f32 CPU matmul as closely as possible.
    try:
        import jax
        import jax.numpy as jnp

        cpu = jax.devices("cpu")[0]
        with jax.default_device(cpu):
            return np.asarray(jnp.matmul(jnp.asarray(x), jnp.asarray(gate_w)))
    except Exception:
        return (x @ gate_w).astype(np.float32)


def _route(x, gate_w, e_bias):
    logits = _gate_logits(x, gate_w)  # [T, E] f32
    scores = (1.0 / (1.0 + np.exp(-logits))).astype(np.float32)
    sfc = scores + e_bias[None, :]
    grp = sfc.reshape(T, N_GROUP, E // N_GROUP)
    group_scores = np.sort(grp, axis=-1)[:, :, -2:].sum(-1)  # [T, G]
    group_idx = np.argsort(-group_scores, axis=-1, kind="stable")[:, :TOPK_GROUP]
    group_mask = np.zeros((T, N_GROUP), bool)
    group_mask[np.arange(T)[:, None], group_idx] = True
    expert_mask = np.repeat(group_mask, E // N_GROUP, axis=1)
    masked = np.where(expert_mask, sfc, -np.inf)
    topk_idx = np.argsort(-masked, axis=-1, kind="stable")[:, :TOP_K]  # [T, 4]
    topk_w = np.take_along_axis(scores, topk_idx, axis=1)
    topk_w = topk_w / topk_w.sum(axis=1, keepdims=True)
    return topk_idx.astype(np.int64), topk_w.astype(np.float32)


# ---------------------------------------------------------------- program
_PROGRAM_CACHE = {}


def _mm3(nc, ps, wt, xt, nk, tok, sz, first, last, level=0, pm=False, ne=0):
    """fp8 DoubleRow contraction over nk k-slices of 128.

    wt: stationary tile [P, nk, 2, P] with slot0=hi, slot1=lo.
    xt: moving tile [P, nk, 2, C] with slot0=lo, slot1=hi — or, with
    pm=True (plane-major), [P, 2, nk, C] with plane0=lo, plane1=hi so the
    hi plane can be DMA'd compactly ahead of the lo plane.
    ps: psum [P, NCH] (use [:, :sz]); tok = token offset into xt.

    level 0: 3-term exact (hi.hi pairs + per-slice full cross terms);
    level 1: weights-exact 2-term (hi.hi + lo.hi pairs), drops w_hi.x_lo;
    level 2: pure hi.hi, additionally drops w_lo.x_hi.
    """

    def mv_hi(j2):  # (2 k-slices, sz) hi rows
        if pm:
            return xt[:, 1, 2 * j2 : 2 * j2 + 2, tok : tok + sz]
        return xt[:, 2 * j2 : 2 * j2 + 2, 1, tok : tok + sz]

    def mv_cross(k):  # ((lo,hi), sz) of one k-slice
        if pm:
            return xt[:, :, k, tok : tok + sz]
        return xt[:, k, :, tok : tok + sz]

    # hi*hi over k-slice pairs
    for j in range(nk // 2):
        nc.tensor.matmul(
            ps[:, :sz],
            wt[:, 2 * j : 2 * j + 2, 0, :],
            mv_hi(j),
            start=(first and j == 0),
            stop=(last and level == 2 and j == nk // 2 - 1),
            perf_mode=DR,
        )
    if level == 0:
        ne = nk
    if level in (0, 1):
        # full cross terms: (w_hi, w_lo) x (x_lo, x_hi) on the ne leading
        # k-slices; weights-exact (w_lo, w_lo') x (x_hi, x_hi') pairs on
        # the rest
        for k in range(ne):
            nc.tensor.matmul(
                ps[:, :sz],
                wt[:, k, :, :],
                mv_cross(k),
                start=False,
                stop=(last and ne == nk and k == nk - 1),
                perf_mode=DR,
            )
        for j in range(ne // 2, nk // 2):
            nc.tensor.matmul(
                ps[:, :sz],
                wt[:, 2 * j : 2 * j + 2, 1, :],
                mv_hi(j),
                start=False,
                stop=(last and j == nk // 2 - 1),
                perf_mode=DR,
            )


def _load_xt(nc, pools, sp, first=False):
    """Emit a spec's x (and wr) loads; idempotent via sp['xt_t']."""
    for th in _xt_load_thunks(nc, pools, sp, first):
        th()


def _xt_load_thunks(nc, pools, sp, first=False):
    """Create the spec's x/wr tiles and return one thunk per DMA, so a
    caller can dribble the emissions between other queue traffic."""
    (xt_pool, w1_pool, w2_pool, g_pool, h_pool, y_pool, wr_pool, sg_pool,
     tmp_pool, ps_gu, ps_dn) = pools
    if "xt_t" in sp:
        return []
    n_d = D // P
    xt_q = nc.gpsimd if sp["bulk_q"] else nc.sync
    C = sp["C"]
    thunks = []
    if first:
        # chunk- and plane-major tile [P, n_chunk, 2, n_d, NCH]: per chunk
        # the hi plane (512KB) lands first so hi.hi passes start early; the
        # lo plane (cross terms) streams behind it
        n_ch = C // NCH
        sp["xt_t"] = xt_pool.tile([P, n_ch, 2, n_d, NCH], DT.float8e4,
                                  name=sp["xt_name"])
        sp["chunk_major"] = True
        for ci in range(n_ch):
            plans = [(1, 0, 4), (1, 4, 16), (0, 0, 8), (0, 8, 16)] if ci == 0 \
                else [(1, 0, 16), (0, 0, 16)]
            for pl, a, b in plans:
                thunks.append(lambda ci=ci, pl=pl, a=a, b=b: xt_q.dma_start(
                    sp["xt_t"][:, ci, pl, a:b, :], sp["xt_h"][ci][:, pl, a:b]))
        return thunks
    sp["xt_t"] = xt_pool.tile([P, n_d, 2, C], DT.float8e4, name=sp["xt_name"])
    if C <= NCH:
        thunks.append(lambda: xt_q.dma_start(sp["xt_t"][:], sp["xt_h"][:, :]))
    else:
        for a, b in zip(range(n_d), range(1, n_d + 1)):
            thunks.append(lambda a=a, b=b: xt_q.dma_start(
                sp["xt_t"][:, a:b, :, :], sp["xt_h"][:, a:b]))
    if sp["apply_wr"]:
        sp["wr_t"] = wr_pool.tile([P, C], DT.float32, name="wr")
        thunks.append(lambda: xt_q.dma_start(sp["wr_t"][:], sp["wr_h"][:, :]))
    return thunks


def _w1_order(n_specs, n_h):
    # merged w1-slice order: per pair ih, each spec's (gate ih, up ih+n_h)
    order = []
    for ih in range(n_h):
        for si in range(n_specs):
            order += [(si, ih, 0), (si, ih + n_h, 1)]
    return order


def _emit_experts(nc, tc, pools, specs, twoI, schemes, first=False,
                  prefetch=(), last=False):
    """Emit 1-2 experts processed interleaved (pair-by-pair, then d2-by-d2).

    Each spec: dict(xt_h, w1_h, w2_h, wr_h, y_h, C, apply_wr, bulk_q).
    A small companion expert rides inside the big one's phases so its
    weight stream amortizes over the long window instead of starving a
    short trailing phase.
    """
    n_d = D // P  # 16 contraction slices over D
    n_i = twoI // P  # gate_up output tiles
    n_h = n_i // 2  # h tiles (= I_/128)

    (xt_pool, w1_pool, w2_pool, g_pool, h_pool, y_pool, wr_pool, sg_pool,
     tmp_pool, ps_gu, ps_dn) = pools

    for sp in specs:
        sp["chunks"] = [(o, min(NCH, sp["C"] - o)) for o in range(0, sp["C"], NCH)]

    order = _w1_order(len(specs), n_h)

    # All w1 loads go on the Pool queue.  Two effects: they never queue
    # behind the previous expert's w2 stream on SP, and — because the queue
    # is in-order and the w1 buffer rotation WAR-throttles it to compute
    # pace — the x bulk loads emitted after them are naturally delayed into
    # the mid-gate_up window, away from the congested phase boundaries.
    def load_w1(si, i):
        t = w1_pool.tile([P, n_d, 2, P], DT.float8e4, name="w1s")
        nc.gpsimd.dma_start(t[:], specs[si]["w1_h"][i])
        return t

    n_pre = 6 if first else 3
    w1_tiles = {j: load_w1(order[j][0], order[j][1]) for j in range(n_pre)}

    # whole-expert X tile [P, k-slice, (lo,hi), tok].  First expert: chunky
    # loads (SP-issue rate is the cold-start limiter).  Later experts: per-d
    # slices on the Pool queue, so each transfer is short and never
    # head-of-line-blocks the latency-critical weight-slice stream on the
    # shared DMA engines.
    for sp in specs:
        _load_xt(nc, pools, sp, first)
        sp["h_t"] = h_pool.tile([P, n_h, 2, sp["C"]], DT.float8e4, name="hil")
        sp["gt"] = {}

    # next experts' x bulk loads dribble into the queue mid-gate_up, two
    # DMAs per pair-step, so they never monopolize the DMA engines against
    # this phase's own weight stream
    pf_thunks = []

    # gate_up: (gate i, up i+n_h) pairs so gate tiles die quickly
    for j, (si, i, half) in enumerate(order):
        if j == n_pre:
            for psp in prefetch:
                pf_thunks += _xt_load_thunks(nc, pools, psp)
        if j >= n_pre:
            for _ in range(2):
                if pf_thunks:
                    pf_thunks.pop(0)()
        sp = specs[si]
        ih = i if half == 0 else i - n_h
        w1s = w1_tiles.pop(j)
        if j + n_pre < len(order):
            nj = j + n_pre
            w1_tiles[nj] = load_w1(order[nj][0], order[nj][1])
        xt_t, h_t = sp["xt_t"], sp["h_t"]
        grp = "g" if half == 0 else "u"
        for ci, (off, sz) in enumerate(sp["chunks"]):
            lvl = schemes.get((sp["slot"], ci, grp), 0)
            lvl, ne = (1, lvl[1]) if isinstance(lvl, tuple) else (lvl, 0)
            ps = ps_gu.tile([P, NCH], DT.float32, name="psg")
            if sp.get("chunk_major"):
                _mm3(nc, ps, w1s, xt_t[:, ci], n_d, 0, sz, True, True, lvl,
                     pm=True, ne=ne)
            else:
                _mm3(nc, ps, w1s, xt_t, n_d, off, sz, True, True, lvl, ne=ne)
            if half == 0:
                sg = sg_pool.tile([P, NCH], DT.float32, name="sg")
                nc.scalar.activation(
                    sg[:, :sz], ps[:, :sz],
                    mybir.ActivationFunctionType.Sigmoid,
                    scale=1.0 / (SX * SW),
                )
                gt = g_pool.tile([P, NCH], DT.float32, name="gt")
                nc.vector.tensor_mul(gt[:, :sz], ps[:, :sz], sg[:, :sz])
                sp["gt"][ci] = gt
            else:
                us = tmp_pool.tile([P, NCH], DT.float32, name="us")
                nc.vector.tensor_scalar_mul(us[:, :sz], ps[:, :sz], CU)
                th = tmp_pool.tile([P, NCH], DT.float32, name="th")
                nc.vector.tensor_mul(th[:, :sz], sp["gt"][ci][:, :sz], us[:, :sz])
                nc.scalar.copy(h_t[:, ih, 1, off : off + sz], th[:, :sz])
                ld = schemes.get((sp["slot"], ci, "d"), 0)
                if ld == 0 or isinstance(ld, tuple):
                    # h_lo only needed where the down-proj runs 3-term
                    df = tmp_pool.tile([P, NCH], DT.float32, name="df")
                    nc.vector.tensor_sub(
                        df[:, :sz], th[:, :sz], h_t[:, ih, 1, off : off + sz]
                    )
                    nc.scalar.copy(h_t[:, ih, 0, off : off + sz], df[:, :sz])

    for th in pf_thunks:
        th()

    # down projection; w2 slices prefetched 3 merged-steps ahead on SP
    dorder = [(si, d2) for d2 in range(D // P) for si in range(len(specs))]

    def load_w2(j, si, d2):
        t = w2_pool.tile([P, n_h, 2, P], DT.float8e4, name="w2s")
        nc.sync.dma_start(t[:], specs[si]["w2_h"][d2])
        return t

    w2_tiles = {j: load_w2(j, *dorder[j]) for j in range(3)}
    for j, (si, d2) in enumerate(dorder):
        sp = specs[si]
        w2s = w2_tiles.pop(j)
        if j + 3 < len(dorder):
            w2_tiles[j + 3] = load_w2(j + 3, *dorder[j + 3])
        chunks, h_t, C = sp["chunks"], sp["h_t"], sp["C"]
        ys = y_pool.tile([P, C], DT.bfloat16, name="ys")
        # one output DMA per row, spread over three queues: HWDGE descriptor
        # generation (one shared unit, ~630ns per DMA) is the down-phase
        # bottleneck, so small rows go to the Pool SWDGE instead
        if C <= NCH:
            y_q = nc.gpsimd
        else:
            y_q = nc.scalar if d2 % 2 else nc.sync
        for ci, (off, sz) in enumerate(chunks):
            lvl = schemes.get((sp["slot"], ci, "d"), 0)
            lvl, ne = (1, lvl[1]) if isinstance(lvl, tuple) else (lvl, 0)
            ps = ps_dn.tile([P, NCH], DT.float32, name="psd")
            _mm3(nc, ps, w2s, h_t, n_h, off, sz, True, True, lvl, ne=ne)
            if sp["apply_wr"]:
                nc.vector.tensor_mul(ys[:, off : off + sz], ps[:, :sz],
                                     sp["wr_t"][:, off : off + sz])
            else:
                nc.scalar.mul(ys[:, off : off + sz], ps[:, :sz], CY)
            if last and j == len(dorder) - 1:
                # final row of the program: drain per-chunk; spread over the
                # two HWDGE queues AND the Pool SWDGE so descriptor
                # generation (625ns apiece, serialized per unit) overlaps
                q = (nc.sync, nc.gpsimd, nc.scalar, nc.gpsimd)[ci % 4]
                q.dma_start(sp["y_h"][d2 * P : (d2 + 1) * P, off : off + sz],
                            ys[:, off : off + sz])
        if not (last and j == len(dorder) - 1):
            y_q.dma_start(sp["y_h"][d2 * P : (d2 + 1) * P, :], ys[:])


def _build_program(C1, C2, C3, skey=()):
    key = (C1, C2, C3, skey)
    if key in _PROGRAM_CACHE:
        return _PROGRAM_CACHE[key]
    schemes = dict(skey)

    nc = bass.Bass(target_bir_lowering=False)
    TS = T // N_CORES  # shared tokens per core
    n_d = D // P

    xt1 = nc.dram_tensor("xt1", [P, n_d, 2, C1], DT.float8e4, kind="ExternalInput")
    xt2 = nc.dram_tensor("xt2", [P, n_d, 2, C2], DT.float8e4, kind="ExternalInput")
    # shared x is chunk-major so the cold start only waits on chunk 0's
    # columns (1MB) instead of the whole tile before the first matmul
    xts = nc.dram_tensor("xts", [TS // NCH, P, 2, n_d, NCH], DT.float8e4,
                         kind="ExternalInput")
    w1a = nc.dram_tensor("w1a", [2 * I // P, P, n_d, 2, P], DT.float8e4, kind="ExternalInput")
    w2a = nc.dram_tensor("w2a", [D // P, P, I // P, 2, P], DT.float8e4, kind="ExternalInput")
    w1b = nc.dram_tensor("w1b", [2 * I // P, P, n_d, 2, P], DT.float8e4, kind="ExternalInput")
    w2b = nc.dram_tensor("w2b", [D // P, P, I // P, 2, P], DT.float8e4, kind="ExternalInput")
    ws1 = nc.dram_tensor("ws1", [2 * SHARED_I // P, P, n_d, 2, P], DT.float8e4, kind="ExternalInput")
    ws2 = nc.dram_tensor("ws2", [D // P, P, SHARED_I // P, 2, P], DT.float8e4, kind="ExternalInput")
    wr1 = nc.dram_tensor("wr1", [P, C1], DT.float32, kind="ExternalInput")
    wr2 = nc.dram_tensor("wr2", [P, C2], DT.float32, kind="ExternalInput")
    y1 = nc.dram_tensor("y1", [D, C1], DT.bfloat16, kind="ExternalOutput")
    y2 = nc.dram_tensor("y2", [D, C2], DT.bfloat16, kind="ExternalOutput")
    ys = nc.dram_tensor("ys", [D, TS], DT.bfloat16, kind="ExternalOutput")
    if C3:
        xt3 = nc.dram_tensor("xt3", [P, n_d, 2, C3], DT.float8e4, kind="ExternalInput")
        w1c = nc.dram_tensor("w1c", [2 * I // P, P, n_d, 2, P], DT.float8e4, kind="ExternalInput")
        w2c = nc.dram_tensor("w2c", [D // P, P, I // P, 2, P], DT.float8e4, kind="ExternalInput")
        wr3 = nc.dram_tensor("wr3", [P, C3], DT.float32, kind="ExternalInput")
        y3 = nc.dram_tensor("y3", [D, C3], DT.bfloat16, kind="ExternalOutput")

    # gate tiles of a pair stay live across all of that pair's chunks: the
    # pool must hold one buffer per chunk or the rotation WAR-deadlocks
    max_chunks = max(-(-c // NCH) for c in (C1, C2, T // N_CORES))
    with tile.TileContext(nc) as tc:
        with (
            tc.tile_pool(name="xt", bufs=1) as xt_pool,
            tc.tile_pool(name="w1p", bufs=6) as w1_pool,
            tc.tile_pool(name="w2p", bufs=4) as w2_pool,
            tc.tile_pool(name="gp", bufs=max(10, max_chunks + 3)) as g_pool,
            tc.tile_pool(name="hp", bufs=2) as h_pool,
            tc.tile_pool(name="yp", bufs=3) as y_pool,
            tc.tile_pool(name="wrp", bufs=2) as wr_pool,
            tc.tile_pool(name="sgp", bufs=4) as sg_pool,
            tc.tile_pool(name="tmp", bufs=4) as tmp_pool,
            tc.tile_pool(name="psgu", bufs=4, space="PSUM") as ps_gu,
            tc.tile_pool(name="psdn", bufs=4, space="PSUM") as ps_dn,
        ):
            pools = (xt_pool, w1_pool, w2_pool, g_pool, h_pool, y_pool, wr_pool,
                     sg_pool, tmp_pool, ps_gu, ps_dn)

            def spec(xt_h, w1_h, w2_h, wr_h, y_h, C, apply_wr, bulk_q, xt_name,
                     slot):
                return dict(xt_h=xt_h, w1_h=w1_h, w2_h=w2_h, wr_h=wr_h,
                            y_h=y_h, C=C, apply_wr=apply_wr, bulk_q=bulk_q,
                            xt_name=xt_name, slot=slot)

            # shared first: its small x-load makes the cold-start short, and
            # the routed experts' larger input streams prefetch underneath it.
            # The small C slot rides inside expert A's phases.
            s_sh = spec(xts, ws1, ws2, None, ys, TS, False, False, "xts", "S")
            sa = [spec(xt1, w1a, w2a, wr1, y1, C1, True, True, "xt1", "A")]
            if C3:
                sa.append(spec(xt3, w1c, w2c, wr3, y3, C3, True, True, "xt3", "C"))
            s_b = spec(xt2, w1b, w2b, wr2, y2, C2, True, True, "xt2", "B")
            _emit_experts(nc, tc, pools, [s_sh], 2 * SHARED_I, schemes,
                          first=True, prefetch=sa)
            _emit_experts(nc, tc, pools, sa, 2 * I, schemes, prefetch=[s_b])
            _emit_experts(nc, tc, pools, [s_b], 2 * I, schemes, last=True)

    _split_excess_waits(nc, limit=1)
    _PROGRAM_CACHE[key] = nc
    return nc


# ---------------------------------------------------------------- packing
def _hi_lo(a, scale):
    s = (a * scale).astype(np.float32)
    hi = s.astype(F8)
    lo = (s - hi.astype(np.float32)).astype(F8)
    return hi, lo


def _pack_w(w, scale):
    """w [K, F] f32 -> [F/P, P(k-in-slice), K/P, 2(hi,lo), P(feat)] e4m3."""
    K, F = w.shape
    n_k, n_f = K // P, F // P
    hi, lo = _hi_lo(w, scale)

    def arr(a):
        return a.reshape(n_k, P, n_f, P).transpose(2, 1, 0, 3)

    out = np.empty((n_f, P, n_k, 2, P), F8)
    out[:, :, :, 0, :] = arr(hi)
    out[:, :, :, 1, :] = arr(lo)
    return np.ascontiguousarray(out)


def _pack_x(xhiT, xloT, cols):
    """xhiT/xloT [D, T] e4m3 + column index -> [P, D/P, 2(lo,hi), C]."""
    n_d = D // P
    C = len(cols)
    out = np.empty((P, n_d, 2, C), F8)
    out[:, :, 0, :] = xloT[:, cols].reshape(n_d, P, C).transpose(1, 0, 2)
    out[:, :, 1, :] = xhiT[:, cols].reshape(n_d, P, C).transpose(1, 0, 2)
    return np.ascontiguousarray(out)


def _cap(n):
    # exact capacity; keep a small floor so degenerate routings stay sane
    return max(P, int(n))


def _plan_slots(counts):
    """Choose slot capacities (alpha, beta, gamma) and the token split.

    Slot A holds the 8 largest experts capped at alpha, slot B the 8
    smallest capped at beta; each expert's overflow goes to one (or more) of
    the 8 per-core C slots of capacity gamma.  Minimizing alpha+beta+gamma
    minimizes the padded per-core column count the SPMD program pays.
    """
    by = np.argsort(-counts, kind="stable")
    big, small = by[:N_CORES], by[N_CORES:]
    cb, cs = counts[big], counts[small]
    def min_gamma(a, b):
        exc = np.concatenate([np.maximum(0, cb - a), np.maximum(0, cs - b)])
        pos = exc[exc > 0]
        if len(pos) == 0:
            return 0
        lo, hi = 1, int(pos.max())
        while lo < hi:
            mid = (lo + hi) // 2
            if np.ceil(pos / mid).sum() <= N_CORES:
                hi = mid
            else:
                lo = mid + 1
        if np.ceil(pos / lo).sum() > N_CORES:
            return None
        return lo

    def scan(a_rng, b_rng):
        best = None
        for a in a_rng:
            for b in b_rng:
                g = min_gamma(a, b)
                if g is None:
                    continue
                tot = a + b + g
                if best is None or tot < best[0]:
                    best = (tot, a, b, g)
        return best

    a_hi, b_hi = int(cb.max()), int(cs.max())
    best = scan(range(max(P, a_hi - 512), a_hi + 1, 8),
                range(max(P, b_hi - 512), b_hi + 1, 8))
    _, a0, b0, _ = best
    best = scan(range(max(P, a0 - 8), min(a_hi, a0 + 8) + 1),
                range(max(P, b0 - 8), min(b_hi, b0 + 8) + 1))
    _, alpha, beta, gamma = best
    # C segments: (expert_id, offset_into_expert_token_list, length)
    segs = []
    for e, cap in [(int(e), alpha) for e in big] + [(int(e), beta) for e in small]:
        exc = int(counts[e]) - cap
        off = cap
        while exc > 0:
            take = min(exc, gamma)
            segs.append((e, off, take))
            off += take
            exc -= take
    assert len(segs) <= N_CORES
    return big, small, alpha, beta, gamma, segs


def _plan_schemes(wlists, C1, C2, C3):
    """Greedy error-budget allocator: per-(slot,chunk,group) matmul levels.

    Each step (exact->2term or 2term->pure) on a routed chunk saves
    32*sz PE cycles and adds VU[g] * (that chunk's share of sum(wr^2))
    of squared-l2 error; columns are wr-ascending with padding first, so
    early chunks are cheap.  Unit variances were measured against the
    f32 reference with single-group probes; pure = 2x 2term (verified).
    Budget keeps the predicted full-output l2 under TARGET (gate 2e-2).

    wlists: slot -> list of per-core padded wr arrays (wr=0 padding)."""
    VU = {"g": 322.9e-6, "u": 301.2e-6, "d": 313.1e-6}
    VS = {"g": 405.6e-6, "u": 378.5e-6, "d": 386.4e-6}
    FLOOR2 = 2.5683e-3 ** 2
    TARGET = 1.979e-2
    W2ALL = max(sum(float((w * w).sum()) for ws in wlists.values() for w in ws),
                1e-30)
    items = []
    for slot, C in (("A", C1), ("B", C2), ("C", C3)):
        if not C:
            continue
        for ci in range(-(-C // NCH)):
            off = ci * NCH
            sz = min(NCH, C - off)
            share = sum(float((w[off : off + sz] ** 2).sum())
                        for w in wlists[slot]) / W2ALL
            for g in ("g", "u", "d"):
                var = max(VU[g] * share, 1e-12)
                for step in (1, 2):
                    items.append((32.0 * sz / var, 32.0 * sz, var, (slot, ci, g), step))
    for g in ("g", "u", "d"):
        for step in (1, 2):
            items.append((64.0 * (T // N_CORES) / VS[g], 64.0 * (T // N_CORES),
                          VS[g], ("S", 0, g), step))
    items.sort(key=lambda t: (-t[0], t[4]))
    budget = TARGET ** 2 - FLOOR2
    taken, used = {}, 0.0
    for _ in range(2):  # second pass lets step-2 follow a same-key step-1
        for ratio, save, var, key, step in items:
            if step != taken.get(key, 0) + 1 or used + var > budget:
                continue
            taken[key] = step
            used += var
    # top-off: the greedy leaves TARGET's conservative margin unspent; the
    # linear variance model tracks measured hardware l2 to ~0.5e-6, so
    # fractional (mixed-ne) steps safely fill up to TOPOFF_TARGET — full
    # cross terms stay on the ne leading k-slices, weights-exact 2-term on
    # the rest
    TOPOFF_TARGET = 1.993e-2
    slack = TOPOFF_TARGET ** 2 - FLOOR2 - used
    for ratio, save, var, key, step in items:
        if slack <= 1e-7:
            break
        if step != 1 or key in taken or key[0] == "S" or var <= 1e-9:
            continue
        nk = 8 if key[2] == "d" else 16
        k_drop = min(int(min(1.0, slack / var) * nk) // 2 * 2, nk - 2)
        if k_drop >= 2:
            taken[key] = ("m", nk - k_drop)
            used += var * k_drop / nk
            slack -= var * k_drop / nk
    return taken


# ---------------------------------------------------------------- kernel
def _prepare(hidden_states, gate_w, e_bias, w_gate_up, w_down, ws_gate_up, ws_down):
    x = np.asarray(hidden_states, dtype=np.float32)
    topk_idx, topk_w = _route(x, np.asarray(gate_w), np.asarray(e_bias))

    # dispatch: token lists per expert, sorted-stable by expert id
    flat_e = topk_idx.ravel()
    order = np.argsort(flat_e, kind="stable")
    pair_tok = order // TOP_K
    pair_w = (topk_w.ravel()[order] * ROUTED_SCALE).astype(np.float32)
    counts = np.bincount(flat_e, minlength=E)
    starts = np.zeros(E + 1, np.int64)
    np.cumsum(counts, out=starts[1:])

    # wr-ascending sort within each expert: low-weight tokens land in the
    # early chunks, where the allocator spends the error budget
    for e in range(E):
        sl = slice(starts[e], starts[e + 1])
        o = np.argsort(pair_w[sl], kind="stable")
        pair_tok[sl] = pair_tok[sl][o]
        pair_w[sl] = pair_w[sl][o]

    # expert -> core assignment: 8 largest in slot A, 8 smallest in slot B
    # (pairing big-with-small per core), overflow segments in slot C
    slotA, slotB_u, alpha, beta, gamma, segs = _plan_slots(counts)
    slotB = slotB_u[::-1]  # pair biggest A with smallest B
    C1 = _cap(alpha)
    C2 = _cap(beta)
    C3 = max(16, int(gamma)) if gamma else 0

    # per-core padded slot arrays, PAD-FIRST: padding columns (wr=0) sit at
    # the low-wr front so the cheap-scheme chunks absorb them for free
    TS = T // N_CORES
    core_slots = []
    wlists = {"A": [], "B": [], "C": []}
    for c in range(N_CORES):
        eA, eB = int(slotA[c]), int(slotB[c])
        segC = segs[c] if c < len(segs) else None
        slots = [("A", eA, 0, C1, C1, "xt1", "wr1"), ("B", eB, 0, C2, C2, "xt2", "wr2")]
        if C3:
            if segC is not None:
                slots.append(("C", segC[0], segC[1], segC[2], C3, "xt3", "wr3"))
            else:
                slots.append(("C", eA, 0, 0, C3, "xt3", "wr3"))
        padded = []
        for slot, e_id, off, cap, C, xt_name, wr_name in slots:
            sl = slice(starts[e_id] + off, min(starts[e_id + 1], starts[e_id] + off + cap))
            idx = pair_tok[sl]
            w = pair_w[sl]
            n_e = len(idx)
            idx_pad = np.zeros(C, np.int64)
            idx_pad[C - n_e :] = idx
            w_pad = np.zeros(C, np.float32)
            w_pad[C - n_e :] = w
            wlists[slot].append(w_pad)
            padded.append((xt_name, wr_name, idx_pad, w_pad, idx, n_e))
        core_slots.append(padded)

    schemes = _plan_schemes(wlists, C1, C2, C3)
    nc = _build_program(C1, C2, C3, tuple(sorted(schemes.items())))

    xhi, xlo = _hi_lo(x, SX)  # [T, D] e4m3
    xhiT = np.ascontiguousarray(xhi.T)  # [D, T]
    xloT = np.ascontiguousarray(xlo.T)

    ws1_p = _pack_w(np.asarray(ws_gate_up), SW)
    ws2_p = _pack_w(np.asarray(ws_down), SW)
    w_gate_up = np.asarray(w_gate_up)
    w_down = np.asarray(w_down)

    in_maps = []
    core_info = []
    zero_w1 = zero_w2 = None
    for c in range(N_CORES):
        eA, eB = int(slotA[c]), int(slotB[c])
        segC = segs[c] if c < len(segs) else None
        m = {}
        info = []
        for xt_name, wr_name, idx_pad, w_pad, idx, n_e in core_slots[c]:
            m[xt_name] = _pack_x(xhiT, xloT, idx_pad)
            m[wr_name] = np.ascontiguousarray(
                np.broadcast_to(w_pad * CY, (P, len(w_pad))))
            info.append((idx, n_e))
        m["xts"] = np.stack([
            np.ascontiguousarray(
                _pack_x(xhiT, xloT,
                        np.arange(c * TS + b * NCH, c * TS + (b + 1) * NCH)
                        ).transpose(0, 2, 1, 3))
            for b in range(TS // NCH)
        ])
        m["w1a"] = _pack_w(w_gate_up[eA], SW)
        m["w2a"] = _pack_w(w_down[eA], SW)
        m["w1b"] = _pack_w(w_gate_up[eB], SW)
        m["w2b"] = _pack_w(w_down[eB], SW)
        if C3:
            if segC is not None:
                m["w1c"] = _pack_w(w_gate_up[segC[0]], SW)
                m["w2c"] = _pack_w(w_down[segC[0]], SW)
            else:
                if zero_w1 is None:
                    zero_w1 = np.zeros((2 * I // P, P, D // P, 2, P), F8)
                    zero_w2 = np.zeros((D // P, P, I // P, 2, P), F8)
                m["w1c"] = zero_w1
                m["w2c"] = zero_w2
        m["ws1"] = ws1_p
        m["ws2"] = ws2_p
        in_maps.append(m)
        core_info.append(info)
    return nc, in_maps, core_info


def _combine(res_results, core_info):
    TS = T // N_CORES
    out = np.zeros((T, D), np.float32)
    for c in range(N_CORES):
        for (idx, n), y_name in zip(core_info[c], ("y1", "y2", "y3")):
            if n:
                # pad-first layout: real columns are the LAST n of the slot
                y = res_results[c][y_name]
                out[idx] += y[:, y.shape[1] - n :].astype(np.float32).T
        out[c * TS : (c + 1) * TS] += res_results[c]["ys"].astype(np.float32).T
    return out


def kernel(hidden_states, gate_w, e_bias, w_gate_up, w_down, ws_gate_up, ws_down):
    nc, in_maps, core_info = _prepare(
        hidden_states, gate_w, e_bias, w_gate_up, w_down, ws_gate_up, ws_down
    )
    res = run_bass_kernel_spmd(nc, in_maps, list(range(N_CORES)))
    return _combine(res.results, core_info)



# revision 42
# speedup vs baseline: 1.0041x; 1.0029x over previous
"""DeepseekV2 MoE layer on 8 Trainium2 NeuronCores.

Strategy (expert-parallel, per the sharding hint):
  - Router gate + grouped top-k computed on host (0.03% of module FLOPs);
    it determines the dispatch, which IS the input sharding.
  - 16 routed experts on 8 cores via three SPMD slots per core: the 8
    largest experts in slot A (capacity alpha), the 8 smallest in slot B
    (beta), and each expert's overflow beyond its slot capacity in a small
    slot C (gamma) on some core.  Capacities are chosen by search to
    minimize alpha+beta+gamma, the padded column count every core pays;
    slot C rides interleaved inside slot A's phases so its weight stream
    amortizes over the long window.
  - Shared-expert MLP is data-parallel over tokens: each core runs
    T/8 = 512 tokens through the full shared MLP.
  - Matmuls run as fp8(e4m3) DoubleRow passes over hi/lo splits of both
    operands.  Per (slot, 256-token chunk, matmul group) a LEVEL is chosen:
      0 = 3-term exact   (1.5 passes/k-slice): hi.hi + both cross terms
      1 = 2-term         (1.0): drops W_hi.X_lo  (weights-exact)
      2 = pure hi.hi     (0.5): also drops W_lo.X_hi
    Each expert's token columns are sorted by routing weight ASCENDING with
    padding first, so cheap levels land where the output barely depends on
    them; _plan_schemes greedily buys the highest cycles-per-variance steps
    until the predicted full-output l2 reaches TARGET=1.96e-2 (gate 2e-2;
    unit variances per group were measured against the f32 reference, and
    the numpy error model reproduces hardware l2 to ~1e-6).
  - f32 PSUM accumulation; bf16 outputs (combined in f32 on host).
"""

import sys

sys.path.insert(0, "/opt/trn_rl_repo")

import copy

import ml_dtypes
import numpy as np

import concourse.bass as bass
import concourse.mybir as mybir
import concourse.tile as tile
from concourse.bass_utils import run_bass_kernel_spmd

DT = mybir.dt
F8 = ml_dtypes.float8_e4m3
BF16 = ml_dtypes.bfloat16
DR = mybir.MatmulPerfMode.DoubleRow

T, D, E, I = 4096, 2048, 16, 1024
TOP_K, N_GROUP, TOPK_GROUP = 4, 4, 2
ROUTED_SCALE = 2.5
SHARED_I = 2048
N_CORES = 8
P = 128
NCH = 256  # token chunk (DoubleRow moving free = 2*NCH = 512 max)

SX = 16.0  # x scale into e4m3
SW = 512.0  # weight scale into e4m3
SH = 8.0  # h scale into e4m3
CU = SH / (SX * SW * SX * SW)  # ps_u -> u*SH/(SX*SW)
CY = 1.0 / (SH * SW)  # down psum descale


# ---------------------------------------------------------------- wait split
def _split_excess_waits(nc, limit=1):
    """This walrus build rejects >1 sync-wait command per instruction.
    Move excess waits onto fresh same-engine NOPs inserted just before."""
    template = bass.Bass(target_bir_lowering=False).sync.nop(nofuse=True).ins
    ctr = 0
    for bb in nc.main_func.blocks:
        out = []
        changed = False
        for ins in bb.instructions:
            si = ins.sync_info
            if si is not None and si.on_wait and len(si.on_wait) > limit:
                waits = list(si.on_wait)
                for w in waits[:-limit]:
                    ctr += 1
                    nop = copy.deepcopy(template)
                    nop.name = f"I-wsplit-{ctr}"
                    nop.engine = ins.engine
                    nop.bass_nofuse = True
                    nop.sync_info = mybir.SyncInfo(on_wait=[w], on_update=[])
                    nc.register_instruction(nop, overwrite=True)
                    out.append(nop)
                ins.sync_info = mybir.SyncInfo(
                    on_wait=waits[-limit:], on_update=list(si.on_update)
                )
                changed = True
            out.append(ins)
        if changed:
            bb.instructions = out
    return ctr


# ---------------------------------------------------------------- routing
def _gate_logits(x, gate_w):
    # Match the reference's jax-f32 CPU matmul as closely as possible.
    try:
        import jax
        import jax.numpy as jnp

        cpu = jax.devices("cpu")[0]
        with jax.default_device(cpu):
            return np.asarray(jnp.matmul(jnp.asarray(x), jnp.asarray(gate_w)))
    except Exception:
        return (x @ gate_w).astype(np.float32)


def _route(x, gate_w, e_bias):
    logits = _gate_logits(x, gate_w)  # [T, E] f32
    scores = (1.0 / (1.0 + np.exp(-logits))).astype(np.float32)
    sfc = scores + e_bias[None, :]
    grp = sfc.reshape(T, N_GROUP, E // N_GROUP)
    group_scores = np.sort(grp, axis=-1)[:, :, -2:].sum(-1)  # [T, G]
    group_idx = np.argsort(-group_scores, axis=-1, kind="stable")[:, :TOPK_GROUP]
    group_mask = np.zeros((T, N_GROUP), bool)
    group_mask[np.arange(T)[:, None], group_idx] = True
    expert_mask = np.repeat(group_mask, E // N_GROUP, axis=1)
    masked = np.where(expert_mask, sfc, -np.inf)
    topk_idx = np.argsort(-masked, axis=-1, kind="stable")[:, :TOP_K]  # [T, 4]
    topk_w = np.take_along_axis(scores, topk_idx, axis=1)
    topk_w = topk_w / topk_w.sum(axis=1, keepdims=True)
    return topk_idx.astype(np.int64), topk_w.astype(np.float32)


# ---------------------------------------------------------------- program
_PROGRAM_CACHE = {}


def _mm3(nc, ps, wt, xt, nk, tok, sz, first, last, level=0, pm=False, ne=0):
    """fp8 DoubleRow contraction over nk k-slices of 128.

    wt: stationary tile [P, nk, 2, P] with slot0=hi, slot1=lo.
    xt: moving tile [P, nk, 2, C] with slot0=lo, slot1=hi — or, with
    pm=True (plane-major), [P, 2, nk, C] with plane0=lo, plane1=hi so the
    hi plane can be DMA'd compactly ahead of the lo plane.
    ps: psum [P, NCH] (use [:, :sz]); tok = token offset into xt.

    level 0: 3-term exact (hi.hi pairs + per-slice full cross terms);
    level 1: weights-exact 2-term (hi.hi + lo.hi pairs), drops w_hi.x_lo;
    level 2: pure hi.hi, additionally drops w_lo.x_hi.
    """

    def mv_hi(j2):  # (2 k-slices, sz) hi rows
        if pm:
            return xt[:, 1, 2 * j2 : 2 * j2 + 2, tok : tok + sz]
        return xt[:, 2 * j2 : 2 * j2 + 2, 1, tok : tok + sz]

    def mv_cross(k):  # ((lo,hi), sz) of one k-slice
        if pm:
            return xt[:, :, k, tok : tok + sz]
        return xt[:, k, :, tok : tok + sz]

    # hi*hi over k-slice pairs
    for j in range(nk // 2):
        nc.tensor.matmul(
            ps[:, :sz],
            wt[:, 2 * j : 2 * j + 2, 0, :],
            mv_hi(j),
            start=(first and j == 0),
            stop=(last and level == 2 and j == nk // 2 - 1),
            perf_mode=DR,
        )
    if level == 0:
        ne = nk
    if level in (0, 1):
        # full cross terms: (w_hi, w_lo) x (x_lo, x_hi) on the ne leading
        # k-slices; weights-exact (w_lo, w_lo') x (x_hi, x_hi') pairs on
        # the rest
        for k in range(ne):
            nc.tensor.matmul(
                ps[:, :sz],
                wt[:, k, :, :],
                mv_cross(k),
                start=False,
                stop=(last and ne == nk and k == nk - 1),
                perf_mode=DR,
            )
        for j in range(ne // 2, nk // 2):
            nc.tensor.matmul(
                ps[:, :sz],
                wt[:, 2 * j : 2 * j + 2, 1, :],
                mv_hi(j),
                start=False,
                stop=(last and j == nk // 2 - 1),
                perf_mode=DR,
            )


def _load_xt(nc, pools, sp, first=False):
    """Emit a spec's x (and wr) loads; idempotent via sp['xt_t']."""
    for th in _xt_load_thunks(nc, pools, sp, first):
        th()


def _xt_load_thunks(nc, pools, sp, first=False):
    """Create the spec's x/wr tiles and return one thunk per DMA, so a
    caller can dribble the emissions between other queue traffic."""
    (xt_pool, w1_pool, w2_pool, g_pool, h_pool, y_pool, wr_pool, sg_pool,
     tmp_pool, ps_gu, ps_dn) = pools
    if "xt_t" in sp:
        return []
    n_d = D // P
    xt_q = nc.gpsimd if sp["bulk_q"] else nc.sync
    C = sp["C"]
    thunks = []
    if first:
        # chunk- and plane-major tile [P, n_chunk, 2, n_d, NCH]: per chunk
        # the hi plane (512KB) lands first so hi.hi passes start early; the
        # lo plane (cross terms) streams behind it
        n_ch = C // NCH
        sp["xt_t"] = xt_pool.tile([P, n_ch, 2, n_d, NCH], DT.float8e4,
                                  name=sp["xt_name"])
        sp["chunk_major"] = True
        for ci in range(n_ch):
            plans = [(1, 0, 4), (1, 4, 16), (0, 0, 8), (0, 8, 16)] if ci == 0 \
                else [(1, 0, 16), (0, 0, 16)]
            for pl, a, b in plans:
                thunks.append(lambda ci=ci, pl=pl, a=a, b=b: xt_q.dma_start(
                    sp["xt_t"][:, ci, pl, a:b, :], sp["xt_h"][ci][:, pl, a:b]))
        return thunks
    sp["xt_t"] = xt_pool.tile([P, n_d, 2, C], DT.float8e4, name=sp["xt_name"])
    if C <= NCH:
        thunks.append(lambda: xt_q.dma_start(sp["xt_t"][:], sp["xt_h"][:, :]))
    else:
        for a, b in zip(range(n_d), range(1, n_d + 1)):
            thunks.append(lambda a=a, b=b: xt_q.dma_start(
                sp["xt_t"][:, a:b, :, :], sp["xt_h"][:, a:b]))
    if sp["apply_wr"]:
        sp["wr_t"] = wr_pool.tile([P, C], DT.float32, name="wr")
        thunks.append(lambda: xt_q.dma_start(sp["wr_t"][:], sp["wr_h"][:, :]))
    return thunks


def _w1_order(n_specs, n_h):
    # merged w1-slice order: per pair ih, each spec's (gate ih, up ih+n_h)
    order = []
    for ih in range(n_h):
        for si in range(n_specs):
            order += [(si, ih, 0), (si, ih + n_h, 1)]
    return order


def _emit_experts(nc, tc, pools, specs, twoI, schemes, first=False,
                  prefetch=(), last=False):
    """Emit 1-2 experts processed interleaved (pair-by-pair, then d2-by-d2).

    Each spec: dict(xt_h, w1_h, w2_h, wr_h, y_h, C, apply_wr, bulk_q).
    A small companion expert rides inside the big one's phases so its
    weight stream amortizes over the long window instead of starving a
    short trailing phase.
    """
    n_d = D // P  # 16 contraction slices over D
    n_i = twoI // P  # gate_up output tiles
    n_h = n_i // 2  # h tiles (= I_/128)

    (xt_pool, w1_pool, w2_pool, g_pool, h_pool, y_pool, wr_pool, sg_pool,
     tmp_pool, ps_gu, ps_dn) = pools

    for sp in specs:
        sp["chunks"] = [(o, min(NCH, sp["C"] - o)) for o in range(0, sp["C"], NCH)]

    order = _w1_order(len(specs), n_h)

    # All w1 loads go on the Pool queue.  Two effects: they never queue
    # behind the previous expert's w2 stream on SP, and — because the queue
    # is in-order and the w1 buffer rotation WAR-throttles it to compute
    # pace — the x bulk loads emitted after them are naturally delayed into
    # the mid-gate_up window, away from the congested phase boundaries.
    def load_w1(si, i):
        t = w1_pool.tile([P, n_d, 2, P], DT.float8e4, name="w1s")
        nc.gpsimd.dma_start(t[:], specs[si]["w1_h"][i])
        return t

    n_pre = 6 if first else 3
    w1_tiles = {j: load_w1(order[j][0], order[j][1]) for j in range(n_pre)}

    # whole-expert X tile [P, k-slice, (lo,hi), tok].  First expert: chunky
    # loads (SP-issue rate is the cold-start limiter).  Later experts: per-d
    # slices on the Pool queue, so each transfer is short and never
    # head-of-line-blocks the latency-critical weight-slice stream on the
    # shared DMA engines.
    for sp in specs:
        _load_xt(nc, pools, sp, first)
        sp["h_t"] = h_pool.tile([P, n_h, 2, sp["C"]], DT.float8e4, name="hil")
        sp["gt"] = {}

    # next experts' x bulk loads dribble into the queue mid-gate_up, two
    # DMAs per pair-step, so they never monopolize the DMA engines against
    # this phase's own weight stream
    pf_thunks = []

    # gate_up: (gate i, up i+n_h) pairs so gate tiles die quickly
    for j, (si, i, half) in enumerate(order):
        if j == n_pre:
            for psp in prefetch:
                pf_thunks += _xt_load_thunks(nc, pools, psp)
        if j >= n_pre:
            for _ in range(2):
                if pf_thunks:
                    pf_thunks.pop(0)()
        sp = specs[si]
        ih = i if half == 0 else i - n_h
        w1s = w1_tiles.pop(j)
        if j + n_pre < len(order):
            nj = j + n_pre
            w1_tiles[nj] = load_w1(order[nj][0], order[nj][1])
        xt_t, h_t = sp["xt_t"], sp["h_t"]
        grp = "g" if half == 0 else "u"
        for ci, (off, sz) in enumerate(sp["chunks"]):
            lvl = schemes.get((sp["slot"], ci, grp), 0)
            lvl, ne = (1, lvl[1]) if isinstance(lvl, tuple) else (lvl, 0)
            ps = ps_gu.tile([P, NCH], DT.float32, name="psg")
            if sp.get("chunk_major"):
                _mm3(nc, ps, w1s, xt_t[:, ci], n_d, 0, sz, True, True, lvl,
                     pm=True, ne=ne)
            else:
                _mm3(nc, ps, w1s, xt_t, n_d, off, sz, True, True, lvl, ne=ne)
            if half == 0:
                sg = sg_pool.tile([P, NCH], DT.float32, name="sg")
                nc.scalar.activation(
                    sg[:, :sz], ps[:, :sz],
                    mybir.ActivationFunctionType.Sigmoid,
                    scale=1.0 / (SX * SW),
                )
                gt = g_pool.tile([P, NCH], DT.float32, name="gt")
                nc.vector.tensor_mul(gt[:, :sz], ps[:, :sz], sg[:, :sz])
                sp["gt"][ci] = gt
            else:
                us = tmp_pool.tile([P, NCH], DT.float32, name="us")
                nc.vector.tensor_scalar_mul(us[:, :sz], ps[:, :sz], CU)
                th = tmp_pool.tile([P, NCH], DT.float32, name="th")
                nc.vector.tensor_mul(th[:, :sz], sp["gt"][ci][:, :sz], us[:, :sz])
                nc.scalar.copy(h_t[:, ih, 1, off : off + sz], th[:, :sz])
                ld = schemes.get((sp["slot"], ci, "d"), 0)
                if ld == 0 or isinstance(ld, tuple):
                    # h_lo only needed where the down-proj runs 3-term
                    df = tmp_pool.tile([P, NCH], DT.float32, name="df")
                    nc.vector.tensor_sub(
                        df[:, :sz], th[:, :sz], h_t[:, ih, 1, off : off + sz]
                    )
                    nc.scalar.copy(h_t[:, ih, 0, off : off + sz], df[:, :sz])

    for th in pf_thunks:
        th()

    # down projection; w2 slices prefetched 3 merged-steps ahead on SP
    dorder = [(si, d2) for d2 in range(D // P) for si in range(len(specs))]

    def load_w2(j, si, d2):
        t = w2_pool.tile([P, n_h, 2, P], DT.float8e4, name="w2s")
        nc.sync.dma_start(t[:], specs[si]["w2_h"][d2])
        return t

    w2_tiles = {j: load_w2(j, *dorder[j]) for j in range(3)}
    for j, (si, d2) in enumerate(dorder):
        sp = specs[si]
        w2s = w2_tiles.pop(j)
        if j + 3 < len(dorder):
            w2_tiles[j + 3] = load_w2(j + 3, *dorder[j + 3])
        chunks, h_t, C = sp["chunks"], sp["h_t"], sp["C"]
        ys = y_pool.tile([P, C], DT.bfloat16, name="ys")
        # one output DMA per row, spread over three queues: HWDGE descriptor
        # generation (one shared unit, ~630ns per DMA) is the down-phase
        # bottleneck, so small rows go to the Pool SWDGE instead
        if C <= NCH:
            y_q = nc.gpsimd
        else:
            y_q = nc.scalar if d2 % 2 else nc.gpsimd
        for ci, (off, sz) in enumerate(chunks):
            lvl = schemes.get((sp["slot"], ci, "d"), 0)
            lvl, ne = (1, lvl[1]) if isinstance(lvl, tuple) else (lvl, 0)
            ps = ps_dn.tile([P, NCH], DT.float32, name="psd")
            _mm3(nc, ps, w2s, h_t, n_h, off, sz, True, True, lvl, ne=ne)
            if sp["apply_wr"]:
                nc.vector.tensor_mul(ys[:, off : off + sz], ps[:, :sz],
                                     sp["wr_t"][:, off : off + sz])
            else:
                nc.scalar.mul(ys[:, off : off + sz], ps[:, :sz], CY)
            if last and j == len(dorder) - 1:
                # final row of the program: drain per-chunk; spread over the
                # two HWDGE queues AND the Pool SWDGE so descriptor
                # generation (625ns apiece, serialized per unit) overlaps
                q = (nc.sync, nc.gpsimd, nc.scalar, nc.gpsimd)[ci % 4]
                q.dma_start(sp["y_h"][d2 * P : (d2 + 1) * P, off : off + sz],
                            ys[:, off : off + sz])
        if not (last and j == len(dorder) - 1):
            y_q.dma_start(sp["y_h"][d2 * P : (d2 + 1) * P, :], ys[:])


def _build_program(C1, C2, C3, skey=()):
    key = (C1, C2, C3, skey)
    if key in _PROGRAM_CACHE:
        return _PROGRAM_CACHE[key]
    schemes = dict(skey)

    nc = bass.Bass(target_bir_lowering=False)
    TS = T // N_CORES  # shared tokens per core
    n_d = D // P

    xt1 = nc.dram_tensor("xt1", [P, n_d, 2, C1], DT.float8e4, kind="ExternalInput")
    xt2 = nc.dram_tensor("xt2", [P, n_d, 2, C2], DT.float8e4, kind="ExternalInput")
    # shared x is chunk-major so the cold start only waits on chunk 0's
    # columns (1MB) instead of the whole tile before the first matmul
    xts = nc.dram_tensor("xts", [TS // NCH, P, 2, n_d, NCH], DT.float8e4,
                         kind="ExternalInput")
    w1a = nc.dram_tensor("w1a", [2 * I // P, P, n_d, 2, P], DT.float8e4, kind="ExternalInput")
    w2a = nc.dram_tensor("w2a", [D // P, P, I // P, 2, P], DT.float8e4, kind="ExternalInput")
    w1b = nc.dram_tensor("w1b", [2 * I // P, P, n_d, 2, P], DT.float8e4, kind="ExternalInput")
    w2b = nc.dram_tensor("w2b", [D // P, P, I // P, 2, P], DT.float8e4, kind="ExternalInput")
    ws1 = nc.dram_tensor("ws1", [2 * SHARED_I // P, P, n_d, 2, P], DT.float8e4, kind="ExternalInput")
    ws2 = nc.dram_tensor("ws2", [D // P, P, SHARED_I // P, 2, P], DT.float8e4, kind="ExternalInput")
    wr1 = nc.dram_tensor("wr1", [P, C1], DT.float32, kind="ExternalInput")
    wr2 = nc.dram_tensor("wr2", [P, C2], DT.float32, kind="ExternalInput")
    y1 = nc.dram_tensor("y1", [D, C1], DT.bfloat16, kind="ExternalOutput")
    y2 = nc.dram_tensor("y2", [D, C2], DT.bfloat16, kind="ExternalOutput")
    ys = nc.dram_tensor("ys", [D, TS], DT.bfloat16, kind="ExternalOutput")
    if C3:
        xt3 = nc.dram_tensor("xt3", [P, n_d, 2, C3], DT.float8e4, kind="ExternalInput")
        w1c = nc.dram_tensor("w1c", [2 * I // P, P, n_d, 2, P], DT.float8e4, kind="ExternalInput")
        w2c = nc.dram_tensor("w2c", [D // P, P, I // P, 2, P], DT.float8e4, kind="ExternalInput")
        wr3 = nc.dram_tensor("wr3", [P, C3], DT.float32, kind="ExternalInput")
        y3 = nc.dram_tensor("y3", [D, C3], DT.bfloat16, kind="ExternalOutput")

    # gate tiles of a pair stay live across all of that pair's chunks: the
    # pool must hold one buffer per chunk or the rotation WAR-deadlocks
    max_chunks = max(-(-c // NCH) for c in (C1, C2, T // N_CORES))
    with tile.TileContext(nc) as tc:
        with (
            tc.tile_pool(name="xt", bufs=1) as xt_pool,
            tc.tile_pool(name="w1p", bufs=6) as w1_pool,
            tc.tile_pool(name="w2p", bufs=4) as w2_pool,
            tc.tile_pool(name="gp", bufs=max(10, max_chunks + 3)) as g_pool,
            tc.tile_pool(name="hp", bufs=2) as h_pool,
            tc.tile_pool(name="yp", bufs=3) as y_pool,
            tc.tile_pool(name="wrp", bufs=2) as wr_pool,
            tc.tile_pool(name="sgp", bufs=4) as sg_pool,
            tc.tile_pool(name="tmp", bufs=4) as tmp_pool,
            tc.tile_pool(name="psgu", bufs=4, space="PSUM") as ps_gu,
            tc.tile_pool(name="psdn", bufs=4, space="PSUM") as ps_dn,
        ):
            pools = (xt_pool, w1_pool, w2_pool, g_pool, h_pool, y_pool, wr_pool,
                     sg_pool, tmp_pool, ps_gu, ps_dn)

            def spec(xt_h, w1_h, w2_h, wr_h, y_h, C, apply_wr, bulk_q, xt_name,
                     slot):
                return dict(xt_h=xt_h, w1_h=w1_h, w2_h=w2_h, wr_h=wr_h,
                            y_h=y_h, C=C, apply_wr=apply_wr, bulk_q=bulk_q,
                            xt_name=xt_name, slot=slot)

            # shared first: its small x-load makes the cold-start short, and
            # the routed experts' larger input streams prefetch underneath it.
            # The small C slot rides inside expert A's phases.
            s_sh = spec(xts, ws1, ws2, None, ys, TS, False, False, "xts", "S")
            sa = [spec(xt1, w1a, w2a, wr1, y1, C1, True, True, "xt1", "A")]
            if C3:
                sa.append(spec(xt3, w1c, w2c, wr3, y3, C3, True, True, "xt3", "C"))
            s_b = spec(xt2, w1b, w2b, wr2, y2, C2, True, True, "xt2", "B")
            _emit_experts(nc, tc, pools, [s_sh], 2 * SHARED_I, schemes,
                          first=True, prefetch=sa)
            _emit_experts(nc, tc, pools, sa, 2 * I, schemes, prefetch=[s_b])
            _emit_experts(nc, tc, pools, [s_b], 2 * I, schemes, last=True)

    _split_excess_waits(nc, limit=1)
    _PROGRAM_CACHE[key] = nc
    return nc


# ---------------------------------------------------------------- packing
def _hi_lo(a, scale):
    s = (a * scale).astype(np.float32)
    hi = s.astype(F8)
    lo = (s - hi.astype(np.float32)).astype(F8)
    return hi, lo


def _pack_w(w, scale):
    """w [K, F] f32 -> [F/P, P(k-in-slice), K/P, 2(hi,lo), P(feat)] e4m3."""
    K, F = w.shape
    n_k, n_f = K // P, F // P
    hi, lo = _hi_lo(w, scale)

    def arr(a):
        return a.reshape(n_k, P, n_f, P).transpose(2, 1, 0, 3)

    out = np.empty((n_f, P, n_k, 2, P), F8)
    out[:, :, :, 0, :] = arr(hi)
    out[:, :, :, 1, :] = arr(lo)
    return np.ascontiguousarray(out)


def _pack_x(xhiT, xloT, cols):
    """xhiT/xloT [D, T] e4m3 + column index -> [P, D/P, 2(lo,hi), C]."""
    n_d = D // P
    C = len(cols)
    out = np.empty((P, n_d, 2, C), F8)
    out[:, :, 0, :] = xloT[:, cols].reshape(n_d, P, C).transpose(1, 0, 2)
    out[:, :, 1, :] = xhiT[:, cols].reshape(n_d, P, C).transpose(1, 0, 2)
    return np.ascontiguousarray(out)


def _cap(n):
    # exact capacity; keep a small floor so degenerate routings stay sane
    return max(P, int(n))


def _plan_slots(counts):
    """Choose slot capacities (alpha, beta, gamma) and the token split.

    Slot A holds the 8 largest experts capped at alpha, slot B the 8
    smallest capped at beta; each expert's overflow goes to one (or more) of
    the 8 per-core C slots of capacity gamma.  Minimizing alpha+beta+gamma
    minimizes the padded per-core column count the SPMD program pays.
    """
    by = np.argsort(-counts, kind="stable")
    big, small = by[:N_CORES], by[N_CORES:]
    cb, cs = counts[big], counts[small]
    def min_gamma(a, b):
        exc = np.concatenate([np.maximum(0, cb - a), np.maximum(0, cs - b)])
        pos = exc[exc > 0]
        if len(pos) == 0:
            return 0
        lo, hi = 1, int(pos.max())
        while lo < hi:
            mid = (lo + hi) // 2
            if np.ceil(pos / mid).sum() <= N_CORES:
                hi = mid
            else:
                lo = mid + 1
        if np.ceil(pos / lo).sum() > N_CORES:
            return None
        return lo

    def scan(a_rng, b_rng):
        best = None
        for a in a_rng:
            for b in b_rng:
                g = min_gamma(a, b)
                if g is None:
                    continue
                tot = a + b + g
                if best is None or tot < best[0]:
                    best = (tot, a, b, g)
        return best

    a_hi, b_hi = int(cb.max()), int(cs.max())
    best = scan(range(max(P, a_hi - 512), a_hi + 1, 8),
                range(max(P, b_hi - 512), b_hi + 1, 8))
    _, a0, b0, _ = best
    best = scan(range(max(P, a0 - 8), min(a_hi, a0 + 8) + 1),
                range(max(P, b0 - 8), min(b_hi, b0 + 8) + 1))
    _, alpha, beta, gamma = best
    # C segments: (expert_id, offset_into_expert_token_list, length)
    segs = []
    for e, cap in [(int(e), alpha) for e in big] + [(int(e), beta) for e in small]:
        exc = int(counts[e]) - cap
        off = cap
        while exc > 0:
            take = min(exc, gamma)
            segs.append((e, off, take))
            off += take
            exc -= take
    assert len(segs) <= N_CORES
    return big, small, alpha, beta, gamma, segs


def _plan_schemes(wlists, C1, C2, C3):
    """Greedy error-budget allocator: per-(slot,chunk,group) matmul levels.

    Each step (exact->2term or 2term->pure) on a routed chunk saves
    32*sz PE cycles and adds VU[g] * (that chunk's share of sum(wr^2))
    of squared-l2 error; columns are wr-ascending with padding first, so
    early chunks are cheap.  Unit variances were measured against the
    f32 reference with single-group probes; pure = 2x 2term (verified).
    Budget keeps the predicted full-output l2 under TARGET (gate 2e-2).

    wlists: slot -> list of per-core padded wr arrays (wr=0 padding)."""
    VU = {"g": 322.9e-6, "u": 301.2e-6, "d": 313.1e-6}
    VS = {"g": 405.6e-6, "u": 378.5e-6, "d": 386.4e-6}
    FLOOR2 = 2.5683e-3 ** 2
    TARGET = 1.979e-2
    W2ALL = max(sum(float((w * w).sum()) for ws in wlists.values() for w in ws),
                1e-30)
    items = []
    for slot, C in (("A", C1), ("B", C2), ("C", C3)):
        if not C:
            continue
        for ci in range(-(-C // NCH)):
            off = ci * NCH
            sz = min(NCH, C - off)
            share = sum(float((w[off : off + sz] ** 2).sum())
                        for w in wlists[slot]) / W2ALL
            for g in ("g", "u", "d"):
                var = max(VU[g] * share, 1e-12)
                for step in (1, 2):
                    items.append((32.0 * sz / var, 32.0 * sz, var, (slot, ci, g), step))
    for g in ("g", "u", "d"):
        for step in (1, 2):
            items.append((64.0 * (T // N_CORES) / VS[g], 64.0 * (T // N_CORES),
                          VS[g], ("S", 0, g), step))
    items.sort(key=lambda t: (-t[0], t[4]))
    budget = TARGET ** 2 - FLOOR2
    taken, used = {}, 0.0
    for _ in range(2):  # second pass lets step-2 follow a same-key step-1
        for ratio, save, var, key, step in items:
            if step != taken.get(key, 0) + 1 or used + var > budget:
                continue
            taken[key] = step
            used += var
    # top-off: the greedy leaves TARGET's conservative margin unspent; the
    # linear variance model tracks measured hardware l2 to ~0.5e-6, so
    # fractional (mixed-ne) steps safely fill up to TOPOFF_TARGET — full
    # cross terms stay on the ne leading k-slices, weights-exact 2-term on
    # the rest
    TOPOFF_TARGET = 1.993e-2
    slack = TOPOFF_TARGET ** 2 - FLOOR2 - used
    for ratio, save, var, key, step in items:
        if slack <= 1e-7:
            break
        if step != 1 or key in taken or key[0] == "S" or var <= 1e-9:
            continue
        nk = 8 if key[2] == "d" else 16
        k_drop = min(int(min(1.0, slack / var) * nk) // 2 * 2, nk - 2)
        if k_drop >= 2:
            taken[key] = ("m", nk - k_drop)
            used += var * k_drop / nk
            slack -= var * k_drop / nk
    return taken


# ---------------------------------------------------------------- kernel
def _prepare(hidden_states, gate_w, e_bias, w_gate_up, w_down, ws_gate_up, ws_down):
    x = np.asarray(hidden_states, dtype=np.float32)
    topk_idx, topk_w = _route(x, np.asarray(gate_w), np.asarray(e_bias))

    # dispatch: token lists per expert, sorted-stable by expert id
    flat_e = topk_idx.ravel()
    order = np.argsort(flat_e, kind="stable")
    pair_tok = order // TOP_K
    pair_w = (topk_w.ravel()[order] * ROUTED_SCALE).astype(np.float32)
    counts = np.bincount(flat_e, minlength=E)
    starts = np.zeros(E + 1, np.int64)
    np.cumsum(counts, out=starts[1:])

    # wr-ascending sort within each expert: low-weight tokens land in the
    # early chunks, where the allocator spends the error budget
    for e in range(E):
        sl = slice(starts[e], starts[e + 1])
        o = np.argsort(pair_w[sl], kind="stable")
        pair_tok[sl] = pair_tok[sl][o]
        pair_w[sl] = pair_w[sl][o]

    # expert -> core assignment: 8 largest in slot A, 8 smallest in slot B
    # (pairing big-with-small per core), overflow segments in slot C
    slotA, slotB_u, alpha, beta, gamma, segs = _plan_slots(counts)
    slotB = slotB_u[::-1]  # pair biggest A with smallest B
    C1 = _cap(alpha)
    C2 = _cap(beta)
    C3 = max(16, int(gamma)) if gamma else 0

    # per-core padded slot arrays, PAD-FIRST: padding columns (wr=0) sit at
    # the low-wr front so the cheap-scheme chunks absorb them for free
    TS = T // N_CORES
    core_slots = []
    wlists = {"A": [], "B": [], "C": []}
    for c in range(N_CORES):
        eA, eB = int(slotA[c]), int(slotB[c])
        segC = segs[c] if c < len(segs) else None
        slots = [("A", eA, 0, C1, C1, "xt1", "wr1"), ("B", eB, 0, C2, C2, "xt2", "wr2")]
        if C3:
            if segC is not None:
                slots.append(("C", segC[0], segC[1], segC[2], C3, "xt3", "wr3"))
            else:
                slots.append(("C", eA, 0, 0, C3, "xt3", "wr3"))
        padded = []
        for slot, e_id, off, cap, C, xt_name, wr_name in slots:
            sl = slice(starts[e_id] + off, min(starts[e_id + 1], starts[e_id] + off + cap))
            idx = pair_tok[sl]
            w = pair_w[sl]
            n_e = len(idx)
            idx_pad = np.zeros(C, np.int64)
            idx_pad[C - n_e :] = idx
            w_pad = np.zeros(C, np.float32)
            w_pad[C - n_e :] = w
            wlists[slot].append(w_pad)
            padded.append((xt_name, wr_name, idx_pad, w_pad, idx, n_e))
        core_slots.append(padded)

    schemes = _plan_schemes(wlists, C1, C2, C3)
    nc = _build_program(C1, C2, C3, tuple(sorted(schemes.items())))

    xhi, xlo = _hi_lo(x, SX)  # [T, D] e4m3
    xhiT = np.ascontiguousarray(xhi.T)  # [D, T]
    xloT = np.ascontiguousarray(xlo.T)

    ws1_p = _pack_w(np.asarray(ws_gate_up), SW)
    ws2_p = _pack_w(np.asarray(ws_down), SW)
    w_gate_up = np.asarray(w_gate_up)
    w_down = np.asarray(w_down)

    in_maps = []
    core_info = []
    zero_w1 = zero_w2 = None
    for c in range(N_CORES):
        eA, eB = int(slotA[c]), int(slotB[c])
        segC = segs[c] if c < len(segs) else None
        m = {}
        info = []
        for xt_name, wr_name, idx_pad, w_pad, idx, n_e in core_slots[c]:
            m[xt_name] = _pack_x(xhiT, xloT, idx_pad)
            m[wr_name] = np.ascontiguousarray(
                np.broadcast_to(w_pad * CY, (P, len(w_pad))))
            info.append((idx, n_e))
        m["xts"] = np.stack([
            np.ascontiguousarray(
                _pack_x(xhiT, xloT,
                        np.arange(c * TS + b * NCH, c * TS + (b + 1) * NCH)
                        ).transpose(0, 2, 1, 3))
            for b in range(TS // NCH)
        ])
        m["w1a"] = _pack_w(w_gate_up[eA], SW)
        m["w2a"] = _pack_w(w_down[eA], SW)
        m["w1b"] = _pack_w(w_gate_up[eB], SW)
        m["w2b"] = _pack_w(w_down[eB], SW)
        if C3:
            if segC is not None:
                m["w1c"] = _pack_w(w_gate_up[segC[0]], SW)
                m["w2c"] = _pack_w(w_down[segC[0]], SW)
            else:
                if zero_w1 is None:
                    zero_w1 = np.zeros((2 * I // P, P, D // P, 2, P), F8)
                    zero_w2 = np.zeros((D // P, P, I // P, 2, P), F8)
                m["w1c"] = zero_w1
                m["w2c"] = zero_w2
        m["ws1"] = ws1_p
        m["ws2"] = ws2_p
        in_maps.append(m)
        core_info.append(info)
    return nc, in_maps, core_info


def _combine(res_results, core_info):
    TS = T // N_CORES
    out = np.zeros((T, D), np.float32)
    for c in range(N_CORES):
        for (idx, n), y_name in zip(core_info[c], ("y1", "y2", "y3")):
            if n:
                # pad-first layout: real columns are the LAST n of the slot
                y = res_results[c][y_name]
                out[idx] += y[:, y.shape[1] - n :].astype(np.float32).T
        out[c * TS : (c + 1) * TS] += res_results[c]["ys"].astype(np.float32).T
    return out


def kernel(hidden_states, gate_w, e_bias, w_gate_up, w_down, ws_gate_up, ws_down):
    nc, in_maps, core_info = _prepare(
        hidden_states, gate_w, e_bias, w_gate_up, w_down, ws_gate_up, ws_down
    )
    res = run_bass_kernel_spmd(nc, in_maps, list(range(N_CORES)))
    return _combine(res.results, core_info)



# revision 52
# speedup vs baseline: 1.0047x; 1.0007x over previous
"""DeepseekV2 MoE layer on 8 Trainium2 NeuronCores.

Strategy (expert-parallel, per the sharding hint):
  - Router gate + grouped top-k computed on host (0.03% of module FLOPs);
    it determines the dispatch, which IS the input sharding.
  - 16 routed experts on 8 cores via three SPMD slots per core: the 8
    largest experts in slot A (capacity alpha), the 8 smallest in slot B
    (beta), and each expert's overflow beyond its slot capacity in a small
    slot C (gamma) on some core.  Capacities are chosen by search to
    minimize alpha+beta+gamma, the padded column count every core pays;
    slot C rides interleaved inside slot A's phases so its weight stream
    amortizes over the long window.
  - Shared-expert MLP is data-parallel over tokens: each core runs
    T/8 = 512 tokens through the full shared MLP.
  - Matmuls run as fp8(e4m3) DoubleRow passes over hi/lo splits of both
    operands.  Per (slot, 256-token chunk, matmul group) a LEVEL is chosen:
      0 = 3-term exact   (1.5 passes/k-slice): hi.hi + both cross terms
      1 = 2-term         (1.0): drops W_hi.X_lo  (weights-exact)
      2 = pure hi.hi     (0.5): also drops W_lo.X_hi
    Each expert's token columns are sorted by routing weight ASCENDING with
    padding first, so cheap levels land where the output barely depends on
    them; _plan_schemes greedily buys the highest cycles-per-variance steps
    until the predicted full-output l2 reaches TARGET=1.96e-2 (gate 2e-2;
    unit variances per group were measured against the f32 reference, and
    the numpy error model reproduces hardware l2 to ~1e-6).
  - f32 PSUM accumulation; bf16 outputs (combined in f32 on host).
"""

import sys

sys.path.insert(0, "/opt/trn_rl_repo")

import copy

import ml_dtypes
import numpy as np

import concourse.bass as bass
import concourse.mybir as mybir
import concourse.tile as tile
from concourse.bass_utils import run_bass_kernel_spmd

DT = mybir.dt
F8 = ml_dtypes.float8_e4m3
BF16 = ml_dtypes.bfloat16
DR = mybir.MatmulPerfMode.DoubleRow

T, D, E, I = 4096, 2048, 16, 1024
TOP_K, N_GROUP, TOPK_GROUP = 4, 4, 2
ROUTED_SCALE = 2.5
SHARED_I = 2048
N_CORES = 8
P = 128
NCH = 256  # token chunk (DoubleRow moving free = 2*NCH = 512 max)

SX = 16.0  # x scale into e4m3
SW = 512.0  # weight scale into e4m3
SH = 8.0  # h scale into e4m3
CU = SH / (SX * SW * SX * SW)  # ps_u -> u*SH/(SX*SW)
CY = 1.0 / (SH * SW)  # down psum descale


# ---------------------------------------------------------------- wait split
def _split_excess_waits(nc, limit=1):
    """This walrus build rejects >1 sync-wait command per instruction.
    Move excess waits onto fresh same-engine NOPs inserted just before."""
    template = bass.Bass(target_bir_lowering=False).sync.nop(nofuse=True).ins
    ctr = 0
    for bb in nc.main_func.blocks:
        out = []
        changed = False
        for ins in bb.instructions:
            si = ins.sync_info
            if si is not None and si.on_wait and len(si.on_wait) > limit:
                waits = list(si.on_wait)
                for w in waits[:-limit]:
                    ctr += 1
                    nop = copy.deepcopy(template)
                    nop.name = f"I-wsplit-{ctr}"
                    nop.engine = ins.engine
                    nop.bass_nofuse = True
                    nop.sync_info = mybir.SyncInfo(on_wait=[w], on_update=[])
                    nc.register_instruction(nop, overwrite=True)
                    out.append(nop)
                ins.sync_info = mybir.SyncInfo(
                    on_wait=waits[-limit:], on_update=list(si.on_update)
                )
                changed = True
            out.append(ins)
        if changed:
            bb.instructions = out
    return ctr


# ---------------------------------------------------------------- routing
def _gate_logits(x, gate_w):
    # Match the reference's jax-f32 CPU matmul as closely as possible.
    try:
        import jax
        import jax.numpy as jnp

        cpu = jax.devices("cpu")[0]
        with jax.default_device(cpu):
            return np.asarray(jnp.matmul(jnp.asarray(x), jnp.asarray(gate_w)))
    except Exception:
        return (x @ gate_w).astype(np.float32)


def _route(x, gate_w, e_bias):
    logits = _gate_logits(x, gate_w)  # [T, E] f32
    scores = (1.0 / (1.0 + np.exp(-logits))).astype(np.float32)
    sfc = scores + e_bias[None, :]
    grp = sfc.reshape(T, N_GROUP, E // N_GROUP)
    group_scores = np.sort(grp, axis=-1)[:, :, -2:].sum(-1)  # [T, G]
    group_idx = np.argsort(-group_scores, axis=-1, kind="stable")[:, :TOPK_GROUP]
    group_mask = np.zeros((T, N_GROUP), bool)
    group_mask[np.arange(T)[:, None], group_idx] = True
    expert_mask = np.repeat(group_mask, E // N_GROUP, axis=1)
    masked = np.where(expert_mask, sfc, -np.inf)
    topk_idx = np.argsort(-masked, axis=-1, kind="stable")[:, :TOP_K]  # [T, 4]
    topk_w = np.take_along_axis(scores, topk_idx, axis=1)
    topk_w = topk_w / topk_w.sum(axis=1, keepdims=True)
    return topk_idx.astype(np.int64), topk_w.astype(np.float32)


# ---------------------------------------------------------------- program
_PROGRAM_CACHE = {}


def _mm3(nc, ps, wt, xt, nk, tok, sz, first, last, level=0, pm=False, ne=0):
    """fp8 DoubleRow contraction over nk k-slices of 128.

    wt: stationary tile [P, nk, 2, P] with slot0=hi, slot1=lo.
    xt: moving tile [P, nk, 2, C] with slot0=lo, slot1=hi — or, with
    pm=True (plane-major), [P, 2, nk, C] with plane0=lo, plane1=hi so the
    hi plane can be DMA'd compactly ahead of the lo plane.
    ps: psum [P, NCH] (use [:, :sz]); tok = token offset into xt.

    level 0: 3-term exact (hi.hi pairs + per-slice full cross terms);
    level 1: weights-exact 2-term (hi.hi + lo.hi pairs), drops w_hi.x_lo;
    level 2: pure hi.hi, additionally drops w_lo.x_hi.
    """

    def mv_hi(j2):  # (2 k-slices, sz) hi rows
        if pm:
            return xt[:, 1, 2 * j2 : 2 * j2 + 2, tok : tok + sz]
        return xt[:, 2 * j2 : 2 * j2 + 2, 1, tok : tok + sz]

    def mv_cross(k):  # ((lo,hi), sz) of one k-slice
        if pm:
            return xt[:, :, k, tok : tok + sz]
        return xt[:, k, :, tok : tok + sz]

    # hi*hi over k-slice pairs
    for j in range(nk // 2):
        nc.tensor.matmul(
            ps[:, :sz],
            wt[:, 2 * j : 2 * j + 2, 0, :],
            mv_hi(j),
            start=(first and j == 0),
            stop=(last and level == 2 and j == nk // 2 - 1),
            perf_mode=DR,
        )
    if level == 0:
        ne = nk
    if level in (0, 1):
        # full cross terms: (w_hi, w_lo) x (x_lo, x_hi) on the ne leading
        # k-slices; weights-exact (w_lo, w_lo') x (x_hi, x_hi') pairs on
        # the rest
        for k in range(ne):
            nc.tensor.matmul(
                ps[:, :sz],
                wt[:, k, :, :],
                mv_cross(k),
                start=False,
                stop=(last and ne == nk and k == nk - 1),
                perf_mode=DR,
            )
        for j in range(ne // 2, nk // 2):
            nc.tensor.matmul(
                ps[:, :sz],
                wt[:, 2 * j : 2 * j + 2, 1, :],
                mv_hi(j),
                start=False,
                stop=(last and j == nk // 2 - 1),
                perf_mode=DR,
            )


def _load_xt(nc, pools, sp, first=False):
    """Emit a spec's x (and wr) loads; idempotent via sp['xt_t']."""
    for th in _xt_load_thunks(nc, pools, sp, first):
        th()


def _xt_load_thunks(nc, pools, sp, first=False):
    """Create the spec's x/wr tiles and return one thunk per DMA, so a
    caller can dribble the emissions between other queue traffic."""
    (xt_pool, w1_pool, w2_pool, g_pool, h_pool, y_pool, wr_pool, sg_pool,
     tmp_pool, ps_gu, ps_dn) = pools
    if "xt_t" in sp:
        return []
    n_d = D // P
    xt_q = nc.gpsimd if sp["bulk_q"] else nc.sync
    C = sp["C"]
    thunks = []
    if first:
        # chunk- and plane-major tile [P, n_chunk, 2, n_d, NCH]: per chunk
        # the hi plane (512KB) lands first so hi.hi passes start early; the
        # lo plane (cross terms) streams behind it
        n_ch = C // NCH
        sp["xt_t"] = xt_pool.tile([P, n_ch, 2, n_d, NCH], DT.float8e4,
                                  name=sp["xt_name"])
        sp["chunk_major"] = True
        for ci in range(n_ch):
            plans = [(1, 0, 4), (1, 4, 16), (0, 0, 8), (0, 8, 16)] if ci == 0 \
                else [(1, 0, 16), (0, 0, 16)]
            for pl, a, b in plans:
                thunks.append(lambda ci=ci, pl=pl, a=a, b=b: xt_q.dma_start(
                    sp["xt_t"][:, ci, pl, a:b, :], sp["xt_h"][ci][:, pl, a:b]))
        return thunks
    sp["xt_t"] = xt_pool.tile([P, n_d, 2, C], DT.float8e4, name=sp["xt_name"])
    if C <= NCH:
        thunks.append(lambda: xt_q.dma_start(sp["xt_t"][:], sp["xt_h"][:, :]))
    else:
        for a, b in zip(range(n_d), range(1, n_d + 1)):
            thunks.append(lambda a=a, b=b: xt_q.dma_start(
                sp["xt_t"][:, a:b, :, :], sp["xt_h"][:, a:b]))
    if sp["apply_wr"]:
        sp["wr_t"] = wr_pool.tile([P, C], DT.float32, name="wr")
        thunks.append(lambda: xt_q.dma_start(sp["wr_t"][:], sp["wr_h"][:, :]))
    return thunks


def _w1_order(n_specs, n_h):
    # merged w1-slice order: per pair ih, each spec's (gate ih, up ih+n_h)
    order = []
    for ih in range(n_h):
        for si in range(n_specs):
            order += [(si, ih, 0), (si, ih + n_h, 1)]
    return order


def _emit_experts(nc, tc, pools, specs, twoI, schemes, first=False,
                  prefetch=(), last=False):
    """Emit 1-2 experts processed interleaved (pair-by-pair, then d2-by-d2).

    Each spec: dict(xt_h, w1_h, w2_h, wr_h, y_h, C, apply_wr, bulk_q).
    A small companion expert rides inside the big one's phases so its
    weight stream amortizes over the long window instead of starving a
    short trailing phase.
    """
    n_d = D // P  # 16 contraction slices over D
    n_i = twoI // P  # gate_up output tiles
    n_h = n_i // 2  # h tiles (= I_/128)

    (xt_pool, w1_pool, w2_pool, g_pool, h_pool, y_pool, wr_pool, sg_pool,
     tmp_pool, ps_gu, ps_dn) = pools

    for sp in specs:
        sp["chunks"] = [(o, min(NCH, sp["C"] - o)) for o in range(0, sp["C"], NCH)]

    order = _w1_order(len(specs), n_h)

    # All w1 loads go on the Pool queue.  Two effects: they never queue
    # behind the previous expert's w2 stream on SP, and — because the queue
    # is in-order and the w1 buffer rotation WAR-throttles it to compute
    # pace — the x bulk loads emitted after them are naturally delayed into
    # the mid-gate_up window, away from the congested phase boundaries.
    def load_w1(si, i):
        t = w1_pool.tile([P, n_d, 2, P], DT.float8e4, name="w1s")
        nc.gpsimd.dma_start(t[:], specs[si]["w1_h"][i])
        return t

    n_pre = 6 if first else 3
    w1_tiles = {j: load_w1(order[j][0], order[j][1]) for j in range(n_pre)}

    # whole-expert X tile [P, k-slice, (lo,hi), tok].  First expert: chunky
    # loads (SP-issue rate is the cold-start limiter).  Later experts: per-d
    # slices on the Pool queue, so each transfer is short and never
    # head-of-line-blocks the latency-critical weight-slice stream on the
    # shared DMA engines.
    for sp in specs:
        _load_xt(nc, pools, sp, first)
        sp["h_t"] = h_pool.tile([P, n_h, 2, sp["C"]], DT.float8e4, name="hil")
        sp["gt"] = {}

    # next experts' x bulk loads dribble into the queue mid-gate_up, two
    # DMAs per pair-step, so they never monopolize the DMA engines against
    # this phase's own weight stream
    pf_thunks = []

    # gate_up: (gate i, up i+n_h) pairs so gate tiles die quickly
    for j, (si, i, half) in enumerate(order):
        if j == n_pre:
            for psp in prefetch:
                pf_thunks += _xt_load_thunks(nc, pools, psp)
        if j >= n_pre:
            for _ in range(2):
                if pf_thunks:
                    pf_thunks.pop(0)()
        sp = specs[si]
        ih = i if half == 0 else i - n_h
        w1s = w1_tiles.pop(j)
        if j + n_pre < len(order):
            nj = j + n_pre
            w1_tiles[nj] = load_w1(order[nj][0], order[nj][1])
        xt_t, h_t = sp["xt_t"], sp["h_t"]
        grp = "g" if half == 0 else "u"
        for ci, (off, sz) in enumerate(sp["chunks"]):
            lvl = schemes.get((sp["slot"], ci, grp), 0)
            lvl, ne = (1, lvl[1]) if isinstance(lvl, tuple) else (lvl, 0)
            ps = ps_gu.tile([P, NCH], DT.float32, name="psg")
            if sp.get("chunk_major"):
                _mm3(nc, ps, w1s, xt_t[:, ci], n_d, 0, sz, True, True, lvl,
                     pm=True, ne=ne)
            else:
                _mm3(nc, ps, w1s, xt_t, n_d, off, sz, True, True, lvl, ne=ne)
            if half == 0:
                sg = sg_pool.tile([P, NCH], DT.float32, name="sg")
                nc.scalar.activation(
                    sg[:, :sz], ps[:, :sz],
                    mybir.ActivationFunctionType.Sigmoid,
                    scale=1.0 / (SX * SW),
                )
                gt = g_pool.tile([P, NCH], DT.float32, name="gt")
                nc.vector.tensor_mul(gt[:, :sz], ps[:, :sz], sg[:, :sz])
                sp["gt"][ci] = gt
            else:
                us = tmp_pool.tile([P, NCH], DT.float32, name="us")
                nc.vector.tensor_scalar_mul(us[:, :sz], ps[:, :sz], CU)
                th = tmp_pool.tile([P, NCH], DT.float32, name="th")
                nc.vector.tensor_mul(th[:, :sz], sp["gt"][ci][:, :sz], us[:, :sz])
                nc.scalar.copy(h_t[:, ih, 1, off : off + sz], th[:, :sz])
                ld = schemes.get((sp["slot"], ci, "d"), 0)
                if ld == 0 or isinstance(ld, tuple):
                    # h_lo only needed where the down-proj runs 3-term
                    df = tmp_pool.tile([P, NCH], DT.float32, name="df")
                    nc.vector.tensor_sub(
                        df[:, :sz], th[:, :sz], h_t[:, ih, 1, off : off + sz]
                    )
                    nc.scalar.copy(h_t[:, ih, 0, off : off + sz], df[:, :sz])

    for th in pf_thunks:
        th()

    # down projection; w2 slices prefetched 3 merged-steps ahead on SP
    dorder = [(si, d2) for d2 in range(D // P) for si in range(len(specs))]

    def load_w2(j, si, d2):
        t = w2_pool.tile([P, n_h, 2, P], DT.float8e4, name="w2s")
        nc.sync.dma_start(t[:], specs[si]["w2_h"][d2])
        return t

    w2_tiles = {j: load_w2(j, *dorder[j]) for j in range(3)}
    for j, (si, d2) in enumerate(dorder):
        sp = specs[si]
        w2s = w2_tiles.pop(j)
        if j + 3 < len(dorder):
            w2_tiles[j + 3] = load_w2(j + 3, *dorder[j + 3])
        chunks, h_t, C = sp["chunks"], sp["h_t"], sp["C"]
        ys = y_pool.tile([P, C], DT.bfloat16, name="ys")
        # one output DMA per row, spread over three queues: HWDGE descriptor
        # generation (one shared unit, ~630ns per DMA) is the down-phase
        # bottleneck, so small rows go to the Pool SWDGE instead
        if C <= NCH:
            y_q = nc.gpsimd
        else:
            y_q = nc.scalar if d2 % 2 else nc.gpsimd
        for ci, (off, sz) in enumerate(chunks):
            lvl = schemes.get((sp["slot"], ci, "d"), 0)
            lvl, ne = (1, lvl[1]) if isinstance(lvl, tuple) else (lvl, 0)
            ps = ps_dn.tile([P, NCH], DT.float32, name="psd")
            _mm3(nc, ps, w2s, h_t, n_h, off, sz, True, True, lvl, ne=ne)
            if sp["apply_wr"]:
                nc.vector.tensor_mul(ys[:, off : off + sz], ps[:, :sz],
                                     sp["wr_t"][:, off : off + sz])
            else:
                nc.scalar.mul(ys[:, off : off + sz], ps[:, :sz], CY)
            if last and j == len(dorder) - 1:
                # final row of the program: drain per-chunk; spread over the
                # two HWDGE queues AND the Pool SWDGE so descriptor
                # generation (625ns apiece, serialized per unit) overlaps
                q = (nc.sync, nc.scalar)[ci % 2]
                q.dma_start(sp["y_h"][d2 * P : (d2 + 1) * P, off : off + sz],
                            ys[:, off : off + sz])
        if not (last and j == len(dorder) - 1):
            y_q.dma_start(sp["y_h"][d2 * P : (d2 + 1) * P, :], ys[:])


def _build_program(C1, C2, C3, skey=()):
    key = (C1, C2, C3, skey)
    if key in _PROGRAM_CACHE:
        return _PROGRAM_CACHE[key]
    schemes = dict(skey)

    nc = bass.Bass(target_bir_lowering=False)
    TS = T // N_CORES  # shared tokens per core
    n_d = D // P

    xt1 = nc.dram_tensor("xt1", [P, n_d, 2, C1], DT.float8e4, kind="ExternalInput")
    xt2 = nc.dram_tensor("xt2", [P, n_d, 2, C2], DT.float8e4, kind="ExternalInput")
    # shared x is chunk-major so the cold start only waits on chunk 0's
    # columns (1MB) instead of the whole tile before the first matmul
    xts = nc.dram_tensor("xts", [TS // NCH, P, 2, n_d, NCH], DT.float8e4,
                         kind="ExternalInput")
    w1a = nc.dram_tensor("w1a", [2 * I // P, P, n_d, 2, P], DT.float8e4, kind="ExternalInput")
    w2a = nc.dram_tensor("w2a", [D // P, P, I // P, 2, P], DT.float8e4, kind="ExternalInput")
    w1b = nc.dram_tensor("w1b", [2 * I // P, P, n_d, 2, P], DT.float8e4, kind="ExternalInput")
    w2b = nc.dram_tensor("w2b", [D // P, P, I // P, 2, P], DT.float8e4, kind="ExternalInput")
    ws1 = nc.dram_tensor("ws1", [2 * SHARED_I // P, P, n_d, 2, P], DT.float8e4, kind="ExternalInput")
    ws2 = nc.dram_tensor("ws2", [D // P, P, SHARED_I // P, 2, P], DT.float8e4, kind="ExternalInput")
    wr1 = nc.dram_tensor("wr1", [P, C1], DT.float32, kind="ExternalInput")
    wr2 = nc.dram_tensor("wr2", [P, C2], DT.float32, kind="ExternalInput")
    y1 = nc.dram_tensor("y1", [D, C1], DT.bfloat16, kind="ExternalOutput")
    y2 = nc.dram_tensor("y2", [D, C2], DT.bfloat16, kind="ExternalOutput")
    ys = nc.dram_tensor("ys", [D, TS], DT.bfloat16, kind="ExternalOutput")
    if C3:
        xt3 = nc.dram_tensor("xt3", [P, n_d, 2, C3], DT.float8e4, kind="ExternalInput")
        w1c = nc.dram_tensor("w1c", [2 * I // P, P, n_d, 2, P], DT.float8e4, kind="ExternalInput")
        w2c = nc.dram_tensor("w2c", [D // P, P, I // P, 2, P], DT.float8e4, kind="ExternalInput")
        wr3 = nc.dram_tensor("wr3", [P, C3], DT.float32, kind="ExternalInput")
        y3 = nc.dram_tensor("y3", [D, C3], DT.bfloat16, kind="ExternalOutput")

    # gate tiles of a pair stay live across all of that pair's chunks: the
    # pool must hold one buffer per chunk or the rotation WAR-deadlocks
    max_chunks = max(-(-c // NCH) for c in (C1, C2, T // N_CORES))
    with tile.TileContext(nc) as tc:
        with (
            tc.tile_pool(name="xt", bufs=1) as xt_pool,
            tc.tile_pool(name="w1p", bufs=6) as w1_pool,
            tc.tile_pool(name="w2p", bufs=4) as w2_pool,
            tc.tile_pool(name="gp", bufs=max(10, max_chunks + 3)) as g_pool,
            tc.tile_pool(name="hp", bufs=2) as h_pool,
            tc.tile_pool(name="yp", bufs=3) as y_pool,
            tc.tile_pool(name="wrp", bufs=2) as wr_pool,
            tc.tile_pool(name="sgp", bufs=4) as sg_pool,
            tc.tile_pool(name="tmp", bufs=4) as tmp_pool,
            tc.tile_pool(name="psgu", bufs=4, space="PSUM") as ps_gu,
            tc.tile_pool(name="psdn", bufs=4, space="PSUM") as ps_dn,
        ):
            pools = (xt_pool, w1_pool, w2_pool, g_pool, h_pool, y_pool, wr_pool,
                     sg_pool, tmp_pool, ps_gu, ps_dn)

            def spec(xt_h, w1_h, w2_h, wr_h, y_h, C, apply_wr, bulk_q, xt_name,
                     slot):
                return dict(xt_h=xt_h, w1_h=w1_h, w2_h=w2_h, wr_h=wr_h,
                            y_h=y_h, C=C, apply_wr=apply_wr, bulk_q=bulk_q,
                            xt_name=xt_name, slot=slot)

            # shared first: its small x-load makes the cold-start short, and
            # the routed experts' larger input streams prefetch underneath it.
            # The small C slot rides inside expert A's phases.
            s_sh = spec(xts, ws1, ws2, None, ys, TS, False, False, "xts", "S")
            sa = [spec(xt1, w1a, w2a, wr1, y1, C1, True, True, "xt1", "A")]
            if C3:
                sa.append(spec(xt3, w1c, w2c, wr3, y3, C3, True, True, "xt3", "C"))
            s_b = spec(xt2, w1b, w2b, wr2, y2, C2, True, True, "xt2", "B")
            _emit_experts(nc, tc, pools, [s_sh], 2 * SHARED_I, schemes,
                          first=True, prefetch=sa)
            _emit_experts(nc, tc, pools, sa, 2 * I, schemes, prefetch=[s_b])
            _emit_experts(nc, tc, pools, [s_b], 2 * I, schemes, last=True)

    _split_excess_waits(nc, limit=1)
    _PROGRAM_CACHE[key] = nc
    return nc


# ---------------------------------------------------------------- packing
def _hi_lo(a, scale):
    s = (a * scale).astype(np.float32)
    hi = s.astype(F8)
    lo = (s - hi.astype(np.float32)).astype(F8)
    return hi, lo


def _pack_w(w, scale):
    """w [K, F] f32 -> [F/P, P(k-in-slice), K/P, 2(hi,lo), P(feat)] e4m3."""
    K, F = w.shape
    n_k, n_f = K // P, F // P
    hi, lo = _hi_lo(w, scale)

    def arr(a):
        return a.reshape(n_k, P, n_f, P).transpose(2, 1, 0, 3)

    out = np.empty((n_f, P, n_k, 2, P), F8)
    out[:, :, :, 0, :] = arr(hi)
    out[:, :, :, 1, :] = arr(lo)
    return np.ascontiguousarray(out)


def _pack_x(xhiT, xloT, cols):
    """xhiT/xloT [D, T] e4m3 + column index -> [P, D/P, 2(lo,hi), C]."""
    n_d = D // P
    C = len(cols)
    out = np.empty((P, n_d, 2, C), F8)
    out[:, :, 0, :] = xloT[:, cols].reshape(n_d, P, C).transpose(1, 0, 2)
    out[:, :, 1, :] = xhiT[:, cols].reshape(n_d, P, C).transpose(1, 0, 2)
    return np.ascontiguousarray(out)


def _cap(n):
    # exact capacity; keep a small floor so degenerate routings stay sane
    return max(P, int(n))


def _plan_slots(counts):
    """Choose slot capacities (alpha, beta, gamma) and the token split.

    Slot A holds the 8 largest experts capped at alpha, slot B the 8
    smallest capped at beta; each expert's overflow goes to one (or more) of
    the 8 per-core C slots of capacity gamma.  Minimizing alpha+beta+gamma
    minimizes the padded per-core column count the SPMD program pays.
    """
    by = np.argsort(-counts, kind="stable")
    big, small = by[:N_CORES], by[N_CORES:]
    cb, cs = counts[big], counts[small]
    def min_gamma(a, b):
        exc = np.concatenate([np.maximum(0, cb - a), np.maximum(0, cs - b)])
        pos = exc[exc > 0]
        if len(pos) == 0:
            return 0
        lo, hi = 1, int(pos.max())
        while lo < hi:
            mid = (lo + hi) // 2
            if np.ceil(pos / mid).sum() <= N_CORES:
                hi = mid
            else:
                lo = mid + 1
        if np.ceil(pos / lo).sum() > N_CORES:
            return None
        return lo

    def scan(a_rng, b_rng):
        best = None
        for a in a_rng:
            for b in b_rng:
                g = min_gamma(a, b)
                if g is None:
                    continue
                tot = a + b + g
                if best is None or tot < best[0]:
                    best = (tot, a, b, g)
        return best

    a_hi, b_hi = int(cb.max()), int(cs.max())
    best = scan(range(max(P, a_hi - 512), a_hi + 1, 8),
                range(max(P, b_hi - 512), b_hi + 1, 8))
    _, a0, b0, _ = best
    best = scan(range(max(P, a0 - 8), min(a_hi, a0 + 8) + 1),
                range(max(P, b0 - 8), min(b_hi, b0 + 8) + 1))
    _, alpha, beta, gamma = best
    # C segments: (expert_id, offset_into_expert_token_list, length)
    segs = []
    for e, cap in [(int(e), alpha) for e in big] + [(int(e), beta) for e in small]:
        exc = int(counts[e]) - cap
        off = cap
        while exc > 0:
            take = min(exc, gamma)
            segs.append((e, off, take))
            off += take
            exc -= take
    assert len(segs) <= N_CORES
    return big, small, alpha, beta, gamma, segs


def _plan_schemes(wlists, C1, C2, C3):
    """Greedy error-budget allocator: per-(slot,chunk,group) matmul levels.

    Each step (exact->2term or 2term->pure) on a routed chunk saves
    32*sz PE cycles and adds VU[g] * (that chunk's share of sum(wr^2))
    of squared-l2 error; columns are wr-ascending with padding first, so
    early chunks are cheap.  Unit variances were measured against the
    f32 reference with single-group probes; pure = 2x 2term (verified).
    Budget keeps the predicted full-output l2 under TARGET (gate 2e-2).

    wlists: slot -> list of per-core padded wr arrays (wr=0 padding)."""
    VU = {"g": 322.9e-6, "u": 301.2e-6, "d": 313.1e-6}
    VS = {"g": 405.6e-6, "u": 378.5e-6, "d": 386.4e-6}
    FLOOR2 = 2.5683e-3 ** 2
    TARGET = 1.979e-2
    W2ALL = max(sum(float((w * w).sum()) for ws in wlists.values() for w in ws),
                1e-30)
    items = []
    for slot, C in (("A", C1), ("B", C2), ("C", C3)):
        if not C:
            continue
        for ci in range(-(-C // NCH)):
            off = ci * NCH
            sz = min(NCH, C - off)
            share = sum(float((w[off : off + sz] ** 2).sum())
                        for w in wlists[slot]) / W2ALL
            for g in ("g", "u", "d"):
                var = max(VU[g] * share, 1e-12)
                for step in (1, 2):
                    items.append((32.0 * sz / var, 32.0 * sz, var, (slot, ci, g), step))
    for g in ("g", "u", "d"):
        for step in (1, 2):
            items.append((64.0 * (T // N_CORES) / VS[g], 64.0 * (T // N_CORES),
                          VS[g], ("S", 0, g), step))
    items.sort(key=lambda t: (-t[0], t[4]))
    budget = TARGET ** 2 - FLOOR2
    taken, used = {}, 0.0
    for _ in range(2):  # second pass lets step-2 follow a same-key step-1
        for ratio, save, var, key, step in items:
            if step != taken.get(key, 0) + 1 or used + var > budget:
                continue
            taken[key] = step
            used += var
    # top-off: the greedy leaves TARGET's conservative margin unspent; the
    # linear variance model tracks measured hardware l2 to ~0.5e-6, so
    # fractional (mixed-ne) steps safely fill up to TOPOFF_TARGET — full
    # cross terms stay on the ne leading k-slices, weights-exact 2-term on
    # the rest
    TOPOFF_TARGET = 1.993e-2
    slack = TOPOFF_TARGET ** 2 - FLOOR2 - used
    for ratio, save, var, key, step in items:
        if slack <= 1e-7:
            break
        if step != 1 or key in taken or key[0] == "S" or var <= 1e-9:
            continue
        nk = 8 if key[2] == "d" else 16
        k_drop = min(int(min(1.0, slack / var) * nk) // 2 * 2, nk - 2)
        if k_drop >= 2:
            taken[key] = ("m", nk - k_drop)
            used += var * k_drop / nk
            slack -= var * k_drop / nk
    return taken


# ---------------------------------------------------------------- kernel
def _prepare(hidden_states, gate_w, e_bias, w_gate_up, w_down, ws_gate_up, ws_down):
    x = np.asarray(hidden_states, dtype=np.float32)
    topk_idx, topk_w = _route(x, np.asarray(gate_w), np.asarray(e_bias))

    # dispatch: token lists per expert, sorted-stable by expert id
    flat_e = topk_idx.ravel()
    order = np.argsort(flat_e, kind="stable")
    pair_tok = order // TOP_K
    pair_w = (topk_w.ravel()[order] * ROUTED_SCALE).astype(np.float32)
    counts = np.bincount(flat_e, minlength=E)
    starts = np.zeros(E + 1, np.int64)
    np.cumsum(counts, out=starts[1:])

    # wr-ascending sort within each expert: low-weight tokens land in the
    # early chunks, where the allocator spends the error budget
    for e in range(E):
        sl = slice(starts[e], starts[e + 1])
        o = np.argsort(pair_w[sl], kind="stable")
        pair_tok[sl] = pair_tok[sl][o]
        pair_w[sl] = pair_w[sl][o]

    # expert -> core assignment: 8 largest in slot A, 8 smallest in slot B
    # (pairing big-with-small per core), overflow segments in slot C
    slotA, slotB_u, alpha, beta, gamma, segs = _plan_slots(counts)
    slotB = slotB_u[::-1]  # pair biggest A with smallest B
    C1 = _cap(alpha)
    C2 = _cap(beta)
    C3 = max(16, int(gamma)) if gamma else 0

    # per-core padded slot arrays, PAD-FIRST: padding columns (wr=0) sit at
    # the low-wr front so the cheap-scheme chunks absorb them for free
    TS = T // N_CORES
    core_slots = []
    wlists = {"A": [], "B": [], "C": []}
    for c in range(N_CORES):
        eA, eB = int(slotA[c]), int(slotB[c])
        segC = segs[c] if c < len(segs) else None
        slots = [("A", eA, 0, C1, C1, "xt1", "wr1"), ("B", eB, 0, C2, C2, "xt2", "wr2")]
        if C3:
            if segC is not None:
                slots.append(("C", segC[0], segC[1], segC[2], C3, "xt3", "wr3"))
            else:
                slots.append(("C", eA, 0, 0, C3, "xt3", "wr3"))
        padded = []
        for slot, e_id, off, cap, C, xt_name, wr_name in slots:
            sl = slice(starts[e_id] + off, min(starts[e_id + 1], starts[e_id] + off + cap))
            idx = pair_tok[sl]
            w = pair_w[sl]
            n_e = len(idx)
            idx_pad = np.zeros(C, np.int64)
            idx_pad[C - n_e :] = idx
            w_pad = np.zeros(C, np.float32)
            w_pad[C - n_e :] = w
            wlists[slot].append(w_pad)
            padded.append((xt_name, wr_name, idx_pad, w_pad, idx, n_e))
        core_slots.append(padded)

    schemes = _plan_schemes(wlists, C1, C2, C3)
    nc = _build_program(C1, C2, C3, tuple(sorted(schemes.items())))

    xhi, xlo = _hi_lo(x, SX)  # [T, D] e4m3
    xhiT = np.ascontiguousarray(xhi.T)  # [D, T]
    xloT = np.ascontiguousarray(xlo.T)

    ws1_p = _pack_w(np.asarray(ws_gate_up), SW)
    ws2_p = _pack_w(np.asarray(ws_down), SW)
    w_gate_up = np.asarray(w_gate_up)
    w_down = np.asarray(w_down)

    in_maps = []
    core_info = []
    zero_w1 = zero_w2 = None
    for c in range(N_CORES):
        eA, eB = int(slotA[c]), int(slotB[c])
        segC = segs[c] if c < len(segs) else None
        m = {}
        info = []
        for xt_name, wr_name, idx_pad, w_pad, idx, n_e in core_slots[c]:
            m[xt_name] = _pack_x(xhiT, xloT, idx_pad)
            m[wr_name] = np.ascontiguousarray(
                np.broadcast_to(w_pad * CY, (P, len(w_pad))))
            info.append((idx, n_e))
        m["xts"] = np.stack([
            np.ascontiguousarray(
                _pack_x(xhiT, xloT,
                        np.arange(c * TS + b * NCH, c * TS + (b + 1) * NCH)
                        ).transpose(0, 2, 1, 3))
            for b in range(TS // NCH)
        ])
        m["w1a"] = _pack_w(w_gate_up[eA], SW)
        m["w2a"] = _pack_w(w_down[eA], SW)
        m["w1b"] = _pack_w(w_gate_up[eB], SW)
        m["w2b"] = _pack_w(w_down[eB], SW)
        if C3:
            if segC is not None:
                m["w1c"] = _pack_w(w_gate_up[segC[0]], SW)
                m["w2c"] = _pack_w(w_down[segC[0]], SW)
            else:
                if zero_w1 is None:
                    zero_w1 = np.zeros((2 * I // P, P, D // P, 2, P), F8)
                    zero_w2 = np.zeros((D // P, P, I // P, 2, P), F8)
                m["w1c"] = zero_w1
                m["w2c"] = zero_w2
        m["ws1"] = ws1_p
        m["ws2"] = ws2_p
        in_maps.append(m)
        core_info.append(info)
    return nc, in_maps, core_info


def _combine(res_results, core_info):
    TS = T // N_CORES
    out = np.zeros((T, D), np.float32)
    for c in range(N_CORES):
        for (idx, n), y_name in zip(core_info[c], ("y1", "y2", "y3")):
            if n:
                # pad-first layout: real columns are the LAST n of the slot
                y = res_results[c][y_name]
                out[idx] += y[:, y.shape[1] - n :].astype(np.float32).T
        out[c * TS : (c + 1) * TS] += res_results[c]["ys"].astype(np.float32).T
    return out


def kernel(hidden_states, gate_w, e_bias, w_gate_up, w_down, ws_gate_up, ws_down):
    nc, in_maps, core_info = _prepare(
        hidden_states, gate_w, e_bias, w_gate_up, w_down, ws_gate_up, ws_down
    )
    res = run_bass_kernel_spmd(nc, in_maps, list(range(N_CORES)))
    return _combine(res.results, core_info)



# revision 55
# speedup vs baseline: 1.0107x; 1.0060x over previous
"""DeepseekV2 MoE layer on 8 Trainium2 NeuronCores.

Strategy (expert-parallel, per the sharding hint):
  - Router gate + grouped top-k computed on host (0.03% of module FLOPs);
    it determines the dispatch, which IS the input sharding.
  - 16 routed experts on 8 cores via three SPMD slots per core: the 8
    largest experts in slot A (capacity alpha), the 8 smallest in slot B
    (beta), and each expert's overflow beyond its slot capacity in a small
    slot C (gamma) on some core.  Capacities are chosen by search to
    minimize alpha+beta+gamma, the padded column count every core pays;
    slot C rides interleaved inside slot A's phases so its weight stream
    amortizes over the long window.
  - Shared-expert MLP is data-parallel over tokens: each core runs
    T/8 = 512 tokens through the full shared MLP.
  - Matmuls run as fp8(e4m3) DoubleRow passes over hi/lo splits of both
    operands.  Per (slot, 256-token chunk, matmul group) a LEVEL is chosen:
      0 = 3-term exact   (1.5 passes/k-slice): hi.hi + both cross terms
      1 = 2-term         (1.0): drops W_hi.X_lo  (weights-exact)
      2 = pure hi.hi     (0.5): also drops W_lo.X_hi
    Each expert's token columns are sorted by routing weight ASCENDING with
    padding first, so cheap levels land where the output barely depends on
    them; _plan_schemes greedily buys the highest cycles-per-variance steps
    until the predicted full-output l2 reaches TARGET=1.96e-2 (gate 2e-2;
    unit variances per group were measured against the f32 reference, and
    the numpy error model reproduces hardware l2 to ~1e-6).
  - f32 PSUM accumulation; bf16 outputs (combined in f32 on host).
"""

import sys

sys.path.insert(0, "/opt/trn_rl_repo")

import copy

import ml_dtypes
import numpy as np

import concourse.bass as bass
import concourse.mybir as mybir
import concourse.tile as tile
from concourse.bass_utils import run_bass_kernel_spmd

DT = mybir.dt
F8 = ml_dtypes.float8_e4m3
BF16 = ml_dtypes.bfloat16
DR = mybir.MatmulPerfMode.DoubleRow

T, D, E, I = 4096, 2048, 16, 1024
TOP_K, N_GROUP, TOPK_GROUP = 4, 4, 2
ROUTED_SCALE = 2.5
SHARED_I = 2048
N_CORES = 8
P = 128
NCH = 256  # token chunk (DoubleRow moving free = 2*NCH = 512 max)

SX = 16.0  # x scale into e4m3
SW = 512.0  # weight scale into e4m3
SH = 8.0  # h scale into e4m3
CU = SH / (SX * SW * SX * SW)  # ps_u -> u*SH/(SX*SW)
CU2 = SH / (SX * SW)  # ps_u -> u*SH (gate side is plain silu(g) via Silu act)
CY = 1.0 / (SH * SW)  # down psum descale


# ---------------------------------------------------------------- wait split
def _split_excess_waits(nc, limit=1):
    """This walrus build rejects >1 sync-wait command per instruction.
    Move excess waits onto fresh same-engine NOPs inserted just before."""
    template = bass.Bass(target_bir_lowering=False).sync.nop(nofuse=True).ins
    ctr = 0
    for bb in nc.main_func.blocks:
        out = []
        changed = False
        for ins in bb.instructions:
            si = ins.sync_info
            if si is not None and si.on_wait and len(si.on_wait) > limit:
                waits = list(si.on_wait)
                for w in waits[:-limit]:
                    ctr += 1
                    nop = copy.deepcopy(template)
                    nop.name = f"I-wsplit-{ctr}"
                    nop.engine = ins.engine
                    nop.bass_nofuse = True
                    nop.sync_info = mybir.SyncInfo(on_wait=[w], on_update=[])
                    nc.register_instruction(nop, overwrite=True)
                    out.append(nop)
                ins.sync_info = mybir.SyncInfo(
                    on_wait=waits[-limit:], on_update=list(si.on_update)
                )
                changed = True
            out.append(ins)
        if changed:
            bb.instructions = out
    return ctr


# ---------------------------------------------------------------- routing
def _gate_logits(x, gate_w):
    # Match the reference's jax-f32 CPU matmul as closely as possible.
    try:
        import jax
        import jax.numpy as jnp

        cpu = jax.devices("cpu")[0]
        with jax.default_device(cpu):
            return np.asarray(jnp.matmul(jnp.asarray(x), jnp.asarray(gate_w)))
    except Exception:
        return (x @ gate_w).astype(np.float32)


def _route(x, gate_w, e_bias):
    logits = _gate_logits(x, gate_w)  # [T, E] f32
    scores = (1.0 / (1.0 + np.exp(-logits))).astype(np.float32)
    sfc = scores + e_bias[None, :]
    grp = sfc.reshape(T, N_GROUP, E // N_GROUP)
    group_scores = np.sort(grp, axis=-1)[:, :, -2:].sum(-1)  # [T, G]
    group_idx = np.argsort(-group_scores, axis=-1, kind="stable")[:, :TOPK_GROUP]
    group_mask = np.zeros((T, N_GROUP), bool)
    group_mask[np.arange(T)[:, None], group_idx] = True
    expert_mask = np.repeat(group_mask, E // N_GROUP, axis=1)
    masked = np.where(expert_mask, sfc, -np.inf)
    topk_idx = np.argsort(-masked, axis=-1, kind="stable")[:, :TOP_K]  # [T, 4]
    topk_w = np.take_along_axis(scores, topk_idx, axis=1)
    topk_w = topk_w / topk_w.sum(axis=1, keepdims=True)
    return topk_idx.astype(np.int64), topk_w.astype(np.float32)


# ---------------------------------------------------------------- program
_PROGRAM_CACHE = {}


def _mm3(nc, ps, wt, xt, nk, tok, sz, first, last, level=0, pm=False, ne=0):
    """fp8 DoubleRow contraction over nk k-slices of 128.

    wt: stationary tile [P, nk, 2, P] with slot0=hi, slot1=lo.
    xt: moving tile [P, nk, 2, C] with slot0=lo, slot1=hi — or, with
    pm=True (plane-major), [P, 2, nk, C] with plane0=lo, plane1=hi so the
    hi plane can be DMA'd compactly ahead of the lo plane.
    ps: psum [P, NCH] (use [:, :sz]); tok = token offset into xt.

    level 0: 3-term exact (hi.hi pairs + per-slice full cross terms);
    level 1: weights-exact 2-term (hi.hi + lo.hi pairs), drops w_hi.x_lo;
    level 2: pure hi.hi, additionally drops w_lo.x_hi.
    """

    def mv_hi(j2):  # (2 k-slices, sz) hi rows
        if pm:
            return xt[:, 1, 2 * j2 : 2 * j2 + 2, tok : tok + sz]
        return xt[:, 2 * j2 : 2 * j2 + 2, 1, tok : tok + sz]

    def mv_cross(k):  # ((lo,hi), sz) of one k-slice
        if pm:
            return xt[:, :, k, tok : tok + sz]
        return xt[:, k, :, tok : tok + sz]

    # hi*hi over k-slice pairs
    for j in range(nk // 2):
        nc.tensor.matmul(
            ps[:, :sz],
            wt[:, 2 * j : 2 * j + 2, 0, :],
            mv_hi(j),
            start=(first and j == 0),
            stop=(last and level == 2 and j == nk // 2 - 1),
            perf_mode=DR,
        )
    if level == 0:
        ne = nk
    if level in (0, 1):
        # full cross terms: (w_hi, w_lo) x (x_lo, x_hi) on the ne leading
        # k-slices; weights-exact (w_lo, w_lo') x (x_hi, x_hi') pairs on
        # the rest
        for k in range(ne):
            nc.tensor.matmul(
                ps[:, :sz],
                wt[:, k, :, :],
                mv_cross(k),
                start=False,
                stop=(last and ne == nk and k == nk - 1),
                perf_mode=DR,
            )
        for j in range(ne // 2, nk // 2):
            nc.tensor.matmul(
                ps[:, :sz],
                wt[:, 2 * j : 2 * j + 2, 1, :],
                mv_hi(j),
                start=False,
                stop=(last and j == nk // 2 - 1),
                perf_mode=DR,
            )


def _load_xt(nc, pools, sp, first=False):
    """Emit a spec's x (and wr) loads; idempotent via sp['xt_t']."""
    for th in _xt_load_thunks(nc, pools, sp, first):
        th()


def _xt_load_thunks(nc, pools, sp, first=False):
    """Create the spec's x/wr tiles and return one thunk per DMA, so a
    caller can dribble the emissions between other queue traffic."""
    (xt_pool, w1_pool, w2_pool, g_pool, h_pool, y_pool, wr_pool, sg_pool,
     tmp_pool, ps_gu, ps_dn) = pools
    if "xt_t" in sp:
        return []
    n_d = D // P
    xt_q = nc.gpsimd if sp["bulk_q"] else nc.sync
    C = sp["C"]
    thunks = []
    if first:
        # chunk- and plane-major tile [P, n_chunk, 2, n_d, NCH]: per chunk
        # the hi plane (512KB) lands first so hi.hi passes start early; the
        # lo plane (cross terms) streams behind it
        n_ch = C // NCH
        sp["xt_t"] = xt_pool.tile([P, n_ch, 2, n_d, NCH], DT.float8e4,
                                  name=sp["xt_name"])
        sp["chunk_major"] = True
        for ci in range(n_ch):
            plans = [(1, 0, 4), (1, 4, 16), (0, 0, 8), (0, 8, 16)] if ci == 0 \
                else [(1, 0, 16), (0, 0, 16)]
            for pl, a, b in plans:
                thunks.append(lambda ci=ci, pl=pl, a=a, b=b: xt_q.dma_start(
                    sp["xt_t"][:, ci, pl, a:b, :], sp["xt_h"][ci][:, pl, a:b]))
        return thunks
    sp["xt_t"] = xt_pool.tile([P, n_d, 2, C], DT.float8e4, name=sp["xt_name"])
    if C <= NCH:
        thunks.append(lambda: xt_q.dma_start(sp["xt_t"][:], sp["xt_h"][:, :]))
    else:
        for a, b in zip(range(n_d), range(1, n_d + 1)):
            thunks.append(lambda a=a, b=b: xt_q.dma_start(
                sp["xt_t"][:, a:b, :, :], sp["xt_h"][:, a:b]))
    if sp["apply_wr"]:
        sp["wr_t"] = wr_pool.tile([P, C], DT.float32, name="wr")
        thunks.append(lambda: xt_q.dma_start(sp["wr_t"][:], sp["wr_h"][:, :]))
    return thunks


def _w1_order(n_specs, n_h):
    # merged w1-slice order: per pair ih, each spec's (gate ih, up ih+n_h)
    order = []
    for ih in range(n_h):
        for si in range(n_specs):
            order += [(si, ih, 0), (si, ih + n_h, 1)]
    return order


def _emit_experts(nc, tc, pools, specs, twoI, schemes, first=False,
                  prefetch=(), last=False):
    """Emit 1-2 experts processed interleaved (pair-by-pair, then d2-by-d2).

    Each spec: dict(xt_h, w1_h, w2_h, wr_h, y_h, C, apply_wr, bulk_q).
    A small companion expert rides inside the big one's phases so its
    weight stream amortizes over the long window instead of starving a
    short trailing phase.
    """
    n_d = D // P  # 16 contraction slices over D
    n_i = twoI // P  # gate_up output tiles
    n_h = n_i // 2  # h tiles (= I_/128)

    (xt_pool, w1_pool, w2_pool, g_pool, h_pool, y_pool, wr_pool, sg_pool,
     tmp_pool, ps_gu, ps_dn) = pools

    for sp in specs:
        sp["chunks"] = [(o, min(NCH, sp["C"] - o)) for o in range(0, sp["C"], NCH)]

    order = _w1_order(len(specs), n_h)

    # All w1 loads go on the Pool queue.  Two effects: they never queue
    # behind the previous expert's w2 stream on SP, and — because the queue
    # is in-order and the w1 buffer rotation WAR-throttles it to compute
    # pace — the x bulk loads emitted after them are naturally delayed into
    # the mid-gate_up window, away from the congested phase boundaries.
    def load_w1(si, i):
        t = w1_pool.tile([P, n_d, 2, P], DT.float8e4, name="w1s")
        nc.gpsimd.dma_start(t[:], specs[si]["w1_h"][i])
        return t

    n_pre = 6 if first else 3
    w1_tiles = {j: load_w1(order[j][0], order[j][1]) for j in range(n_pre)}

    # whole-expert X tile [P, k-slice, (lo,hi), tok].  First expert: chunky
    # loads (SP-issue rate is the cold-start limiter).  Later experts: per-d
    # slices on the Pool queue, so each transfer is short and never
    # head-of-line-blocks the latency-critical weight-slice stream on the
    # shared DMA engines.
    for sp in specs:
        _load_xt(nc, pools, sp, first)
        sp["h_t"] = h_pool.tile([P, n_h, 2, sp["C"]], DT.float8e4, name="hil")
        sp["gt"] = {}

    # next experts' x bulk loads dribble into the queue mid-gate_up, two
    # DMAs per pair-step, so they never monopolize the DMA engines against
    # this phase's own weight stream
    pf_thunks = []

    # gate_up: (gate i, up i+n_h) pairs so gate tiles die quickly
    for j, (si, i, half) in enumerate(order):
        if j == n_pre:
            for psp in prefetch:
                pf_thunks += _xt_load_thunks(nc, pools, psp)
        if j >= n_pre:
            for _ in range(2):
                if pf_thunks:
                    pf_thunks.pop(0)()
        sp = specs[si]
        ih = i if half == 0 else i - n_h
        w1s = w1_tiles.pop(j)
        if j + n_pre < len(order):
            nj = j + n_pre
            w1_tiles[nj] = load_w1(order[nj][0], order[nj][1])
        xt_t, h_t = sp["xt_t"], sp["h_t"]
        grp = "g" if half == 0 else "u"
        for ci, (off, sz) in enumerate(sp["chunks"]):
            lvl = schemes.get((sp["slot"], ci, grp), 0)
            lvl, ne = (1, lvl[1]) if isinstance(lvl, tuple) else (lvl, 0)
            ps = ps_gu.tile([P, NCH], DT.float32, name="psg")
            if sp.get("chunk_major"):
                _mm3(nc, ps, w1s, xt_t[:, ci], n_d, 0, sz, True, True, lvl,
                     pm=True, ne=ne)
            else:
                _mm3(nc, ps, w1s, xt_t, n_d, off, sz, True, True, lvl, ne=ne)
            if half == 0:
                # single Silu activation: gt = silu(ps/(SX*SW)) = silu(g);
                # the SX*SW factor the old ps*sigmoid carried moves into the
                # up-half descale (CU2), so th = silu(g)*u*SH is unchanged
                gt = g_pool.tile([P, NCH], DT.float32, name="gt")
                nc.scalar.activation(
                    gt[:, :sz], ps[:, :sz],
                    mybir.ActivationFunctionType.Silu,
                    scale=1.0 / (SX * SW),
                )
                sp["gt"][ci] = gt
            else:
                us = tmp_pool.tile([P, NCH], DT.float32, name="us")
                nc.vector.tensor_scalar_mul(us[:, :sz], ps[:, :sz], CU2)
                th = tmp_pool.tile([P, NCH], DT.float32, name="th")
                nc.vector.tensor_mul(th[:, :sz], sp["gt"][ci][:, :sz], us[:, :sz])
                nc.scalar.copy(h_t[:, ih, 1, off : off + sz], th[:, :sz])
                ld = schemes.get((sp["slot"], ci, "d"), 0)
                if ld == 0 or isinstance(ld, tuple):
                    # h_lo only needed where the down-proj runs 3-term
                    df = tmp_pool.tile([P, NCH], DT.float32, name="df")
                    nc.vector.tensor_sub(
                        df[:, :sz], th[:, :sz], h_t[:, ih, 1, off : off + sz]
                    )
                    nc.scalar.copy(h_t[:, ih, 0, off : off + sz], df[:, :sz])

    for th in pf_thunks:
        th()

    # down projection; w2 slices prefetched 3 merged-steps ahead on SP
    dorder = [(si, d2) for d2 in range(D // P) for si in range(len(specs))]

    def load_w2(j, si, d2):
        t = w2_pool.tile([P, n_h, 2, P], DT.float8e4, name="w2s")
        nc.sync.dma_start(t[:], specs[si]["w2_h"][d2])
        return t

    w2_tiles = {j: load_w2(j, *dorder[j]) for j in range(3)}
    for j, (si, d2) in enumerate(dorder):
        sp = specs[si]
        w2s = w2_tiles.pop(j)
        if j + 3 < len(dorder):
            w2_tiles[j + 3] = load_w2(j + 3, *dorder[j + 3])
        chunks, h_t, C = sp["chunks"], sp["h_t"], sp["C"]
        ys = y_pool.tile([P, C], DT.bfloat16, name="ys")
        # one output DMA per row, spread over three queues: HWDGE descriptor
        # generation (one shared unit, ~630ns per DMA) is the down-phase
        # bottleneck, so small rows go to the Pool SWDGE instead
        if C <= NCH:
            y_q = nc.gpsimd
        else:
            y_q = nc.scalar if d2 % 2 else nc.gpsimd
        for ci, (off, sz) in enumerate(chunks):
            lvl = schemes.get((sp["slot"], ci, "d"), 0)
            lvl, ne = (1, lvl[1]) if isinstance(lvl, tuple) else (lvl, 0)
            ps = ps_dn.tile([P, NCH], DT.float32, name="psd")
            _mm3(nc, ps, w2s, h_t, n_h, off, sz, True, True, lvl, ne=ne)
            if sp["apply_wr"]:
                nc.vector.tensor_mul(ys[:, off : off + sz], ps[:, :sz],
                                     sp["wr_t"][:, off : off + sz])
            else:
                nc.scalar.mul(ys[:, off : off + sz], ps[:, :sz], CY)
            if last and j == len(dorder) - 1:
                # final row of the program: drain per-chunk; spread over the
                # two HWDGE queues AND the Pool SWDGE so descriptor
                # generation (625ns apiece, serialized per unit) overlaps
                q = (nc.sync, nc.scalar)[ci % 2]
                q.dma_start(sp["y_h"][d2 * P : (d2 + 1) * P, off : off + sz],
                            ys[:, off : off + sz])
        if not (last and j == len(dorder) - 1):
            y_q.dma_start(sp["y_h"][d2 * P : (d2 + 1) * P, :], ys[:])


def _build_program(C1, C2, C3, skey=()):
    key = (C1, C2, C3, skey)
    if key in _PROGRAM_CACHE:
        return _PROGRAM_CACHE[key]
    schemes = dict(skey)

    nc = bass.Bass(target_bir_lowering=False)
    TS = T // N_CORES  # shared tokens per core
    n_d = D // P

    xt1 = nc.dram_tensor("xt1", [P, n_d, 2, C1], DT.float8e4, kind="ExternalInput")
    xt2 = nc.dram_tensor("xt2", [P, n_d, 2, C2], DT.float8e4, kind="ExternalInput")
    # shared x is chunk-major so the cold start only waits on chunk 0's
    # columns (1MB) instead of the whole tile before the first matmul
    xts = nc.dram_tensor("xts", [TS // NCH, P, 2, n_d, NCH], DT.float8e4,
                         kind="ExternalInput")
    w1a = nc.dram_tensor("w1a", [2 * I // P, P, n_d, 2, P], DT.float8e4, kind="ExternalInput")
    w2a = nc.dram_tensor("w2a", [D // P, P, I // P, 2, P], DT.float8e4, kind="ExternalInput")
    w1b = nc.dram_tensor("w1b", [2 * I // P, P, n_d, 2, P], DT.float8e4, kind="ExternalInput")
    w2b = nc.dram_tensor("w2b", [D // P, P, I // P, 2, P], DT.float8e4, kind="ExternalInput")
    ws1 = nc.dram_tensor("ws1", [2 * SHARED_I // P, P, n_d, 2, P], DT.float8e4, kind="ExternalInput")
    ws2 = nc.dram_tensor("ws2", [D // P, P, SHARED_I // P, 2, P], DT.float8e4, kind="ExternalInput")
    wr1 = nc.dram_tensor("wr1", [P, C1], DT.float32, kind="ExternalInput")
    wr2 = nc.dram_tensor("wr2", [P, C2], DT.float32, kind="ExternalInput")
    y1 = nc.dram_tensor("y1", [D, C1], DT.bfloat16, kind="ExternalOutput")
    y2 = nc.dram_tensor("y2", [D, C2], DT.bfloat16, kind="ExternalOutput")
    ys = nc.dram_tensor("ys", [D, TS], DT.bfloat16, kind="ExternalOutput")
    if C3:
        xt3 = nc.dram_tensor("xt3", [P, n_d, 2, C3], DT.float8e4, kind="ExternalInput")
        w1c = nc.dram_tensor("w1c", [2 * I // P, P, n_d, 2, P], DT.float8e4, kind="ExternalInput")
        w2c = nc.dram_tensor("w2c", [D // P, P, I // P, 2, P], DT.float8e4, kind="ExternalInput")
        wr3 = nc.dram_tensor("wr3", [P, C3], DT.float32, kind="ExternalInput")
        y3 = nc.dram_tensor("y3", [D, C3], DT.bfloat16, kind="ExternalOutput")

    # gate tiles of a pair stay live across all of that pair's chunks: the
    # pool must hold one buffer per chunk or the rotation WAR-deadlocks
    max_chunks = max(-(-c // NCH) for c in (C1, C2, T // N_CORES))
    with tile.TileContext(nc) as tc:
        with (
            tc.tile_pool(name="xt", bufs=1) as xt_pool,
            tc.tile_pool(name="w1p", bufs=6) as w1_pool,
            tc.tile_pool(name="w2p", bufs=4) as w2_pool,
            tc.tile_pool(name="gp", bufs=max(10, max_chunks + 3)) as g_pool,
            tc.tile_pool(name="hp", bufs=2) as h_pool,
            tc.tile_pool(name="yp", bufs=3) as y_pool,
            tc.tile_pool(name="wrp", bufs=2) as wr_pool,
            tc.tile_pool(name="sgp", bufs=4) as sg_pool,
            tc.tile_pool(name="tmp", bufs=4) as tmp_pool,
            tc.tile_pool(name="psgu", bufs=4, space="PSUM") as ps_gu,
            tc.tile_pool(name="psdn", bufs=4, space="PSUM") as ps_dn,
        ):
            pools = (xt_pool, w1_pool, w2_pool, g_pool, h_pool, y_pool, wr_pool,
                     sg_pool, tmp_pool, ps_gu, ps_dn)

            def spec(xt_h, w1_h, w2_h, wr_h, y_h, C, apply_wr, bulk_q, xt_name,
                     slot):
                return dict(xt_h=xt_h, w1_h=w1_h, w2_h=w2_h, wr_h=wr_h,
                            y_h=y_h, C=C, apply_wr=apply_wr, bulk_q=bulk_q,
                            xt_name=xt_name, slot=slot)

            # shared first: its small x-load makes the cold-start short, and
            # the routed experts' larger input streams prefetch underneath it.
            # The small C slot rides inside expert A's phases.
            s_sh = spec(xts, ws1, ws2, None, ys, TS, False, False, "xts", "S")
            sa = [spec(xt1, w1a, w2a, wr1, y1, C1, True, True, "xt1", "A")]
            if C3:
                sa.append(spec(xt3, w1c, w2c, wr3, y3, C3, True, True, "xt3", "C"))
            s_b = spec(xt2, w1b, w2b, wr2, y2, C2, True, True, "xt2", "B")
            _emit_experts(nc, tc, pools, [s_sh], 2 * SHARED_I, schemes,
                          first=True, prefetch=sa)
            _emit_experts(nc, tc, pools, sa, 2 * I, schemes, prefetch=[s_b])
            _emit_experts(nc, tc, pools, [s_b], 2 * I, schemes, last=True)

    _split_excess_waits(nc, limit=1)
    _PROGRAM_CACHE[key] = nc
    return nc


# ---------------------------------------------------------------- packing
def _hi_lo(a, scale):
    s = (a * scale).astype(np.float32)
    hi = s.astype(F8)
    lo = (s - hi.astype(np.float32)).astype(F8)
    return hi, lo


def _pack_w(w, scale):
    """w [K, F] f32 -> [F/P, P(k-in-slice), K/P, 2(hi,lo), P(feat)] e4m3."""
    K, F = w.shape
    n_k, n_f = K // P, F // P
    hi, lo = _hi_lo(w, scale)

    def arr(a):
        return a.reshape(n_k, P, n_f, P).transpose(2, 1, 0, 3)

    out = np.empty((n_f, P, n_k, 2, P), F8)
    out[:, :, :, 0, :] = arr(hi)
    out[:, :, :, 1, :] = arr(lo)
    return np.ascontiguousarray(out)


def _pack_x(xhiT, xloT, cols):
    """xhiT/xloT [D, T] e4m3 + column index -> [P, D/P, 2(lo,hi), C]."""
    n_d = D // P
    C = len(cols)
    out = np.empty((P, n_d, 2, C), F8)
    out[:, :, 0, :] = xloT[:, cols].reshape(n_d, P, C).transpose(1, 0, 2)
    out[:, :, 1, :] = xhiT[:, cols].reshape(n_d, P, C).transpose(1, 0, 2)
    return np.ascontiguousarray(out)


def _cap(n):
    # exact capacity; keep a small floor so degenerate routings stay sane
    return max(P, int(n))


def _plan_slots(counts):
    """Choose slot capacities (alpha, beta, gamma) and the token split.

    Slot A holds the 8 largest experts capped at alpha, slot B the 8
    smallest capped at beta; each expert's overflow goes to one (or more) of
    the 8 per-core C slots of capacity gamma.  Minimizing alpha+beta+gamma
    minimizes the padded per-core column count the SPMD program pays.
    """
    by = np.argsort(-counts, kind="stable")
    big, small = by[:N_CORES], by[N_CORES:]
    cb, cs = counts[big], counts[small]
    def min_gamma(a, b):
        exc = np.concatenate([np.maximum(0, cb - a), np.maximum(0, cs - b)])
        pos = exc[exc > 0]
        if len(pos) == 0:
            return 0
        lo, hi = 1, int(pos.max())
        while lo < hi:
            mid = (lo + hi) // 2
            if np.ceil(pos / mid).sum() <= N_CORES:
                hi = mid
            else:
                lo = mid + 1
        if np.ceil(pos / lo).sum() > N_CORES:
            return None
        return lo

    def scan(a_rng, b_rng):
        best = None
        for a in a_rng:
            for b in b_rng:
                g = min_gamma(a, b)
                if g is None:
                    continue
                tot = a + b + g
                if best is None or tot < best[0]:
                    best = (tot, a, b, g)
        return best

    a_hi, b_hi = int(cb.max()), int(cs.max())
    best = scan(range(max(P, a_hi - 512), a_hi + 1, 8),
                range(max(P, b_hi - 512), b_hi + 1, 8))
    _, a0, b0, _ = best
    best = scan(range(max(P, a0 - 8), min(a_hi, a0 + 8) + 1),
                range(max(P, b0 - 8), min(b_hi, b0 + 8) + 1))
    _, alpha, beta, gamma = best
    # C segments: (expert_id, offset_into_expert_token_list, length)
    segs = []
    for e, cap in [(int(e), alpha) for e in big] + [(int(e), beta) for e in small]:
        exc = int(counts[e]) - cap
        off = cap
        while exc > 0:
            take = min(exc, gamma)
            segs.append((e, off, take))
            off += take
            exc -= take
    assert len(segs) <= N_CORES
    return big, small, alpha, beta, gamma, segs


def _plan_schemes(wlists, C1, C2, C3):
    """Greedy error-budget allocator: per-(slot,chunk,group) matmul levels.

    Each step (exact->2term or 2term->pure) on a routed chunk saves
    32*sz PE cycles and adds VU[g] * (that chunk's share of sum(wr^2))
    of squared-l2 error; columns are wr-ascending with padding first, so
    early chunks are cheap.  Unit variances were measured against the
    f32 reference with single-group probes; pure = 2x 2term (verified).
    Budget keeps the predicted full-output l2 under TARGET (gate 2e-2).

    wlists: slot -> list of per-core padded wr arrays (wr=0 padding)."""
    VU = {"g": 322.9e-6, "u": 301.2e-6, "d": 313.1e-6}
    VS = {"g": 405.6e-6, "u": 378.5e-6, "d": 386.4e-6}
    FLOOR2 = 2.5683e-3 ** 2
    TARGET = 1.979e-2
    W2ALL = max(sum(float((w * w).sum()) for ws in wlists.values() for w in ws),
                1e-30)
    items = []
    for slot, C in (("A", C1), ("B", C2), ("C", C3)):
        if not C:
            continue
        for ci in range(-(-C // NCH)):
            off = ci * NCH
            sz = min(NCH, C - off)
            share = sum(float((w[off : off + sz] ** 2).sum())
                        for w in wlists[slot]) / W2ALL
            for g in ("g", "u", "d"):
                var = max(VU[g] * share, 1e-12)
                for step in (1, 2):
                    items.append((32.0 * sz / var, 32.0 * sz, var, (slot, ci, g), step))
    for g in ("g", "u", "d"):
        for step in (1, 2):
            items.append((64.0 * (T // N_CORES) / VS[g], 64.0 * (T // N_CORES),
                          VS[g], ("S", 0, g), step))
    items.sort(key=lambda t: (-t[0], t[4]))
    budget = TARGET ** 2 - FLOOR2
    taken, used = {}, 0.0
    for _ in range(2):  # second pass lets step-2 follow a same-key step-1
        for ratio, save, var, key, step in items:
            if step != taken.get(key, 0) + 1 or used + var > budget:
                continue
            taken[key] = step
            used += var
    # top-off: the greedy leaves TARGET's conservative margin unspent; the
    # linear variance model tracks measured hardware l2 to ~0.5e-6, so
    # fractional (mixed-ne) steps safely fill up to TOPOFF_TARGET — full
    # cross terms stay on the ne leading k-slices, weights-exact 2-term on
    # the rest
    TOPOFF_TARGET = 1.993e-2
    slack = TOPOFF_TARGET ** 2 - FLOOR2 - used
    for ratio, save, var, key, step in items:
        if slack <= 1e-7:
            break
        if step != 1 or key in taken or key[0] == "S" or var <= 1e-9:
            continue
        nk = 8 if key[2] == "d" else 16
        k_drop = min(int(min(1.0, slack / var) * nk) // 2 * 2, nk - 2)
        if k_drop >= 2:
            taken[key] = ("m", nk - k_drop)
            used += var * k_drop / nk
            slack -= var * k_drop / nk
    return taken


# ---------------------------------------------------------------- kernel
def _prepare(hidden_states, gate_w, e_bias, w_gate_up, w_down, ws_gate_up, ws_down):
    x = np.asarray(hidden_states, dtype=np.float32)
    topk_idx, topk_w = _route(x, np.asarray(gate_w), np.asarray(e_bias))

    # dispatch: token lists per expert, sorted-stable by expert id
    flat_e = topk_idx.ravel()
    order = np.argsort(flat_e, kind="stable")
    pair_tok = order // TOP_K
    pair_w = (topk_w.ravel()[order] * ROUTED_SCALE).astype(np.float32)
    counts = np.bincount(flat_e, minlength=E)
    starts = np.zeros(E + 1, np.int64)
    np.cumsum(counts, out=starts[1:])

    # wr-ascending sort within each expert: low-weight tokens land in the
    # early chunks, where the allocator spends the error budget
    for e in range(E):
        sl = slice(starts[e], starts[e + 1])
        o = np.argsort(pair_w[sl], kind="stable")
        pair_tok[sl] = pair_tok[sl][o]
        pair_w[sl] = pair_w[sl][o]

    # expert -> core assignment: 8 largest in slot A, 8 smallest in slot B
    # (pairing big-with-small per core), overflow segments in slot C
    slotA, slotB_u, alpha, beta, gamma, segs = _plan_slots(counts)
    slotB = slotB_u[::-1]  # pair biggest A with smallest B
    C1 = _cap(alpha)
    C2 = _cap(beta)
    C3 = max(16, int(gamma)) if gamma else 0

    # per-core padded slot arrays, PAD-FIRST: padding columns (wr=0) sit at
    # the low-wr front so the cheap-scheme chunks absorb them for free
    TS = T // N_CORES
    core_slots = []
    wlists = {"A": [], "B": [], "C": []}
    for c in range(N_CORES):
        eA, eB = int(slotA[c]), int(slotB[c])
        segC = segs[c] if c < len(segs) else None
        slots = [("A", eA, 0, C1, C1, "xt1", "wr1"), ("B", eB, 0, C2, C2, "xt2", "wr2")]
        if C3:
            if segC is not None:
                slots.append(("C", segC[0], segC[1], segC[2], C3, "xt3", "wr3"))
            else:
                slots.append(("C", eA, 0, 0, C3, "xt3", "wr3"))
        padded = []
        for slot, e_id, off, cap, C, xt_name, wr_name in slots:
            sl = slice(starts[e_id] + off, min(starts[e_id + 1], starts[e_id] + off + cap))
            idx = pair_tok[sl]
            w = pair_w[sl]
            n_e = len(idx)
            idx_pad = np.zeros(C, np.int64)
            idx_pad[C - n_e :] = idx
            w_pad = np.zeros(C, np.float32)
            w_pad[C - n_e :] = w
            wlists[slot].append(w_pad)
            padded.append((xt_name, wr_name, idx_pad, w_pad, idx, n_e))
        core_slots.append(padded)

    schemes = _plan_schemes(wlists, C1, C2, C3)
    nc = _build_program(C1, C2, C3, tuple(sorted(schemes.items())))

    xhi, xlo = _hi_lo(x, SX)  # [T, D] e4m3
    xhiT = np.ascontiguousarray(xhi.T)  # [D, T]
    xloT = np.ascontiguousarray(xlo.T)

    ws1_p = _pack_w(np.asarray(ws_gate_up), SW)
    ws2_p = _pack_w(np.asarray(ws_down), SW)
    w_gate_up = np.asarray(w_gate_up)
    w_down = np.asarray(w_down)

    in_maps = []
    core_info = []
    zero_w1 = zero_w2 = None
    for c in range(N_CORES):
        eA, eB = int(slotA[c]), int(slotB[c])
        segC = segs[c] if c < len(segs) else None
        m = {}
        info = []
        for xt_name, wr_name, idx_pad, w_pad, idx, n_e in core_slots[c]:
            m[xt_name] = _pack_x(xhiT, xloT, idx_pad)
            m[wr_name] = np.ascontiguousarray(
                np.broadcast_to(w_pad * CY, (P, len(w_pad))))
            info.append((idx, n_e))
        m["xts"] = np.stack([
            np.ascontiguousarray(
                _pack_x(xhiT, xloT,
                        np.arange(c * TS + b * NCH, c * TS + (b + 1) * NCH)
                        ).transpose(0, 2, 1, 3))
            for b in range(TS // NCH)
        ])
        m["w1a"] = _pack_w(w_gate_up[eA], SW)
        m["w2a"] = _pack_w(w_down[eA], SW)
        m["w1b"] = _pack_w(w_gate_up[eB], SW)
        m["w2b"] = _pack_w(w_down[eB], SW)
        if C3:
            if segC is not None:
                m["w1c"] = _pack_w(w_gate_up[segC[0]], SW)
                m["w2c"] = _pack_w(w_down[segC[0]], SW)
            else:
                if zero_w1 is None:
                    zero_w1 = np.zeros((2 * I // P, P, D // P, 2, P), F8)
                    zero_w2 = np.zeros((D // P, P, I // P, 2, P), F8)
                m["w1c"] = zero_w1
                m["w2c"] = zero_w2
        m["ws1"] = ws1_p
        m["ws2"] = ws2_p
        in_maps.append(m)
        core_info.append(info)
    return nc, in_maps, core_info


def _combine(res_results, core_info):
    TS = T // N_CORES
    out = np.zeros((T, D), np.float32)
    for c in range(N_CORES):
        for (idx, n), y_name in zip(core_info[c], ("y1", "y2", "y3")):
            if n:
                # pad-first layout: real columns are the LAST n of the slot
                y = res_results[c][y_name]
                out[idx] += y[:, y.shape[1] - n :].astype(np.float32).T
        out[c * TS : (c + 1) * TS] += res_results[c]["ys"].astype(np.float32).T
    return out


def kernel(hidden_states, gate_w, e_bias, w_gate_up, w_down, ws_gate_up, ws_down):
    nc, in_maps, core_info = _prepare(
        hidden_states, gate_w, e_bias, w_gate_up, w_down, ws_gate_up, ws_down
    )
    res = run_bass_kernel_spmd(nc, in_maps, list(range(N_CORES)))
    return _combine(res.results, core_info)

